# revision 91
# baseline (speedup 1.0000x reference)
"""
Bass/Trainium2 kernel for nn_BottleneckShared (moe_routing).

Computation (per sample b):
    rw   = sigmoid(mean_hw(x) @ router_w.T + router_b)          # [E]
    Wk_b = sum_e rw[e] * wk[e]            (k = 1,2,3)           # per-sample conv kernels
    out  = relu(bn3(conv3(relu(bn2(conv2(relu(bn1(conv1(x)))))))) + x)

Sharding: data-parallel over batch. 64 samples -> 8 NeuronCores x 8 samples.
Expert banks / router / BN params are replicated to every core.

Device-side design notes (v3):
 - BN scales folded into the expert banks on the host; BN biases applied in
   the PSUM-drain of each conv.
 - Weight combine via a block-diagonal router matrix streamed against
   STATIONARY bank chunks: bank chunk tc is a [128, 128] tile whose partition
   (e, j in 16) holds bank_e[:, 16*tc + j]; the moving operand is
   R_s[(e,j), j'] = rw_se * delta_jj' ([128, 16]).  Each matmul produces 16
   combined-weight columns directly:
     out[i, j'] = sum_{e,j} bank_e[i, 16tc+j] * rw_se delta_jj'
                = sum_e rw_se bank_e[i, 16tc+j'].
   PE cost is 16 columns per chunk (2176 columns/sample) vs 512 per expert
   (17408/sample) for the scaled-identity formulation.
 - Router computed transposed: rwT[(e,j), s] = sigmoid(sum_i RW4[i,(e,j)] *
   pooled_s[i] + b[e]) so R_s is one tensor_scalar_mul of a [128, 16] mask by
   the per-partition scalar rwT.
 - conv1/conv3 are 1x1 convs = matmuls over the 784 pixels; conv2 (3x3,
   pad 1) = 9 accumulating matmuls with shifted access patterns over a
   zero-padded [128, 30, 30] tile.
 - Residual: identity matmul accumulates x into the conv3 PSUM; bias+relu
   applied in the drains.
 - Engine balance per sample: PE 7.8us (convs+combines+residual+router),
   DVE ~6.9us (2 reduces, combine drains, conv3 c=1 drains), ACT ~6.7us
   (2 reduces via accum_out, sigmoid, conv1/2 drains, conv3 c=0 drains),
   Pool (memset only; GPSIMD cannot touch PSUM and cannot reduce the free
   axis, so it cannot take real work).  Conv PSUM tiles stay small
   ([128, 14, 28], bufs=4) — deep buffering decouples the PE from drain
   latency; pairing banks into wider tiles was tried and made conv3
   drain-paced.
 - Emission is software-pipelined one sample ahead: slot s runs convs of s
   on the PE while emitting the combines of s+1 (drains overlap convs) and
   the router of s+2 at slot end (keeps reduces behind the drains in the
   in-order DVE/ACT queues).  DMA order: x0, cc, b1R, biases, b2 pieces,
   x1, b3R, then remaining x in sample order; outputs trail each sample.
"""

import sys

import ml_dtypes
import numpy as np

sys.path.insert(0, "/opt/trn_rl_repo")

import concourse.bacc as bacc
import concourse.bass as bass
import concourse.mybir as mybir
import concourse.tile as tile
from concourse import bass_utils

EPS = 1e-5

B = 64          # global batch
NCORES = 8
BS = B // NCORES  # samples per core
E = 8           # experts
CIN = 512
WID = 128       # bottleneck width
COUT = 512
H = 28
P = H * H       # 784 pixels
NCH = 392       # pixels per conv output chunk (14 rows)

NC1 = 32        # 16-col combine chunks for W1 (512 cols)
NC2 = 72        # for W2 (1152 cols)
NC3 = 32        # for W3 (512 cols)

F16 = mybir.dt.float16
F32 = mybir.dt.float32
F8 = mybir.dt.float8e4


def build_program():
    nc = bacc.Bacc("TRN2", target_bir_lowering=False, debug=False)

    # ---- DRAM I/O (per-core shapes) ----
    x_d = nc.dram_tensor("x", [BS, 4, 128, P], F16, kind="ExternalInput")
    b1_d = nc.dram_tensor("bank1", [128, NC1 // 2, 2, 128], F8, kind="ExternalInput")
    b2_d = nc.dram_tensor("bank2", [128, NC2 // 2, 2, 128], F8, kind="ExternalInput")
    b3_d = nc.dram_tensor("bank3", [128, NC3 // 2, 2, 128], F8, kind="ExternalInput")
    wbar_d = nc.dram_tensor("wbar", [128, 2176], F16, kind="ExternalInput")
    cc_d = nc.dram_tensor("cc", [128, 896], F16, kind="ExternalInput")
    bias1_d = nc.dram_tensor("bias1", [128, 1], F32, kind="ExternalInput")
    bias2_d = nc.dram_tensor("bias2", [128, 1], F32, kind="ExternalInput")
    bias3_d = nc.dram_tensor("bias3", [128, 4], F32, kind="ExternalInput")
    out_d = nc.dram_tensor("out", [BS, 4, 128, P], F16, kind="ExternalOutput")

    Relu = mybir.ActivationFunctionType.Relu
    Sigmoid = mybir.ActivationFunctionType.Sigmoid
    Copy = mybir.ActivationFunctionType.Copy
    ADD = mybir.AluOpType.add
    MAX = mybir.AluOpType.max

    with tile.TileContext(nc) as tc:
        with (
            tc.tile_pool(name="const", bufs=1) as constp,
            tc.tile_pool(name="xin", bufs=6) as xp,
            tc.tile_pool(name="xsplit", bufs=1) as xsp,
            tc.tile_pool(name="comb", bufs=2) as combp,
            tc.tile_pool(name="act", bufs=4) as actp,
            tc.tile_pool(name="small", bufs=4) as smallp,
            tc.tile_pool(name="rmat", bufs=6) as rp,
            tc.tile_pool(name="resid", bufs=4) as residp,
            tc.tile_pool(name="pscomb", bufs=2, space=bass.MemorySpace.PSUM) as pscombp,
            tc.tile_pool(name="psconv", bufs=5, space=bass.MemorySpace.PSUM) as psconvp,
            tc.tile_pool(name="psr", bufs=1, space=bass.MemorySpace.PSUM) as psrp,
        ):
            # ---- persistent constants ----
            # banks: [(e,j16), tc32, jt, i], 8x scaled fp8 (DoubleRow k-tiles)
            bank1 = constp.tile([128, NC1 // 2, 2, 128], F8)
            bank2 = constp.tile([128, NC2 // 2, 2, 128], F8)
            bank3 = constp.tile([128, NC3 // 2, 2, 128], F8)
            wbar = constp.tile([128, 2176], F16)       # 0.5*sum_e bank_e
            cc = constp.tile([128, 896], F16)
            rw4 = cc[:, 0:512]        # RW4[i, (it,(e,j))]
            mask32 = cc[:, 512:576]   # [(e,j16),(jt,j')] = 16*delta_{16jt+j16,j'}
            ident = cc[:, 576:704]    # I128 for the residual matmul
            rbT = cc[0:1, 704:832]    # row 0: router_b[(e,j)]
            maskneg = cc[:, 832:896]  # -8*delta pattern
            ones1 = constp.tile([1, 1], F16)
            bias1 = constp.tile([128, 1], F32)
            bias2 = constp.tile([128, 1], F32)
            bias3 = constp.tile([128, 4], F32)

            nc.gpsimd.memset(ones1[:], 1.0)

            # DMA issue order = device service order. Sample 0's tiles and
            # the banks go first (they gate sample 0's convs); later samples'
            # x tiles follow the banks in need-order.
            xs_l, r_l = [], []
            for s in (0, 1):
                xs_l.append([
                    xsp.tile([128, P], F16, tag=f"xs{s}_{t}", name=f"xs{s}_{t}")
                    for t in range(4)
                ])
            xs0 = xs_l[0]
            for s in range(2, BS):
                big = xp.tile([128, 4 * P], F16, tag="xs", name=f"xs{s}")
                xs_l.append([big[:, t * P : (t + 1) * P] for t in range(4)])
            nc.sync.dma_start(xs0[0][:], x_d[0, 0])
            nc.sync.dma_start(xs0[1][:], x_d[0, 1])
            nc.sync.dma_start(xs0[2][:], x_d[0, 2])
            nc.sync.dma_start(xs0[3][:], x_d[0, 3])
            nc.sync.dma_start(cc[:], cc_d[:])
            nc.sync.dma_start(xs_l[1][0][:], x_d[1, 0])
            nc.sync.dma_start(xs_l[1][1][:], x_d[1, 1])
            nc.sync.dma_start(xs_l[1][2][:], x_d[1, 2])
            nc.sync.dma_start(xs_l[1][3][:], x_d[1, 3])
            nc.sync.dma_start(bank1[:], b1_d[:])
            nc.sync.dma_start(wbar[:, 0:512], wbar_d[:, 0:512])
            nc.sync.dma_start(bias1[:], bias1_d[:])
            nc.sync.dma_start(bias2[:], bias2_d[:])
            nc.sync.dma_start(bias3[:], bias3_d[:])
            nc.sync.dma_start(bank2[:, 0:16], b2_d[:, 0:16])
            nc.sync.dma_start(wbar[:, 512:1664], wbar_d[:, 512:1664])
            nc.sync.dma_start(bank2[:, 16:32], b2_d[:, 16:32])
            nc.sync.dma_start(bank2[:, 32:36], b2_d[:, 32:36])
            nc.sync.dma_start(
                xs_l[2][0].tensor[:, :], x_d[2].transpose([1, 0, 2])
            )
            nc.sync.dma_start(bank3[:], b3_d[:])
            nc.sync.dma_start(wbar[:, 1664:2176], wbar_d[:, 1664:2176])
            for s in range(3, BS):
                nc.sync.dma_start(
                    xs_l[s][0].tensor[:, :], x_d[s].transpose([1, 0, 2])
                )

            # ================= routers =====================================
            pooled_l = {}

            def emit_router_reduce(s):
                xs = xs_l[s]

                # Pooling split across DVE and ACT (accum_out).  For the
                # front samples (tiles landing 558ns apart) 3 DVE / 1 ACT
                # minimizes the latency to R; in steady state 2/2 balances
                # engine load.
                dve_tiles = (0, 1, 3) if s < 2 else (0, 1)
                pooled = smallp.tile([128, 4], F32, tag="pooled")
                for t in range(4):
                    if t in dve_tiles:
                        nc.vector.tensor_reduce(
                            pooled[:, t : t + 1],
                            xs[t][:, :],
                            axis=mybir.AxisListType.X,
                            op=ADD,
                        )
                    else:
                        scratch = smallp.tile([128, P], F16, tag="scratch")
                        nc.scalar.activation(
                            scratch[:],
                            xs[t][:, :],
                            Copy,
                            accum_out=pooled[:, t : t + 1],
                        )
                pooled_l[s] = pooled

            def emit_router_finish(s):
                pooled = pooled_l.pop(s)
                pooled16 = smallp.tile([128, 4], F16, tag="pooled16")
                nc.vector.tensor_copy(pooled16[:], pooled[:])

                rpsum = psrp.tile([128, 1], F32, tag="rpsum")
                for t in range(4):
                    nc.tensor.matmul(
                        rpsum[:],
                        rw4[:, t * 128 : (t + 1) * 128],
                        pooled16[:, t : t + 1],
                        start=(t == 0),
                        stop=False,
                    )
                nc.tensor.matmul(
                    rpsum[:],
                    rbT,
                    ones1[:],
                    start=False,
                    stop=True,
                )
                rwT = smallp.tile([128, 1], F32, tag="rwT", name=f"rwT{s}")
                nc.scalar.activation(rwT[:], rpsum[:], Sigmoid)

                rs = rp.tile([128, 2, 32], F8, tag="rs", name=f"rs{s}")
                nc.vector.scalar_tensor_tensor(
                    rs[:].rearrange("p t j -> p (t j)"),
                    mask32, rwT[:, 0:1], maskneg,
                    op0=mybir.AluOpType.mult, op1=ADD,
                )
                r_l.append(rs)

            def emit_router(s):
                emit_router_reduce(s)
                emit_router_finish(s)

            # ============ combines (bank chunks stationary) =================
            w_l = {}

            def combine(s, bank, a0, ncols, dst, d0, woff):
                # ncols 32-col chunks; DoubleRow fp8 matmuls contract 256
                # rows ((e, j in 32)) at 0.5 cycles/row
                rs = r_l[s]
                psc = pscombp.tile([128, 512], F32, tag="psc", name="psc")
                for a in range(ncols):
                    nc.tensor.matmul(
                        psc[:, 32 * a : 32 * a + 32],
                        bank[:, a0 + a, :, :],
                        rs[:],
                        start=True,
                        stop=True,
                        perf_mode=mybir.MatmulPerfMode.DoubleRow,
                    )
                # dst = wbar + psc/128  (the drain applies the fp8 scale
                # normalization and adds the expert-mean weights)
                nc.vector.scalar_tensor_tensor(
                    dst[:, d0 : d0 + 32 * ncols],
                    psc[:, : 32 * ncols],
                    1.0 / 128.0,
                    wbar[:, woff + d0 : woff + d0 + 32 * ncols],
                    op0=mybir.AluOpType.mult,
                    op1=ADD,
                )

            def emit_cw1(s):
                w1c = combp.tile([128, 512], F16, tag="w1c", name=f"w1c{s}")
                combine(s, bank1, 0, 16, w1c, 0, 0)
                w_l.setdefault(s, {})["w1"] = w1c

            def emit_cw2(s):
                w2c = combp.tile([128, 1152], F16, tag="w2c", name=f"w2c{s}")
                combine(s, bank2, 0, 16, w2c, 0, 512)
                combine(s, bank2, 16, 16, w2c, 512, 512)
                combine(s, bank2, 32, 4, w2c, 1024, 512)
                w_l.setdefault(s, {})["w2"] = w2c

            def emit_cw3(s):
                w3c = combp.tile([128, 512], F16, tag="w3c", name=f"w3c{s}")
                combine(s, bank3, 0, 16, w3c, 0, 1664)
                w_l.setdefault(s, {})["w3"] = w3c

            # ============ convs =============================================
            def emit_conv1(s):
                xs = xs_l[s]
                w1c = w_l[s]["w1"]
                mid1 = actp.tile([128, 30, 30], F16, tag="mid1", name=f"mid1_{s}")
                nc.gpsimd.memset(mid1[:], 0.0)
                for c in range(2):
                    ps1 = psconvp.tile([128, 14, 28], F32, tag="convps")
                    for k in range(4):
                        nc.tensor.matmul(
                            ps1[:],
                            w1c[:, k * 128 : (k + 1) * 128],
                            xs[k][:, c * NCH : (c + 1) * NCH],
                            start=(k == 0),
                            stop=(k == 3),
                        )
                    nc.scalar.activation(
                        mid1[:, 14 * c + 1 : 14 * c + 15, 1:29],
                        ps1[:],
                        Relu,
                        bias=bias1[:],
                    )
                w_l[s]["mid1"] = mid1

            def emit_conv2(s):
                mid1 = w_l[s]["mid1"]
                w2c = w_l[s]["w2"]
                out2 = actp.tile([128, P], F16, tag="out2", name=f"out2_{s}")
                for c in range(2):
                    ps2 = psconvp.tile([128, 14, 28], F32, tag="convps")
                    idx = 0
                    for dy in range(3):
                        for dx in range(3):
                            nc.tensor.matmul(
                                ps2[:],
                                w2c[:, (dy * 3 + dx) * 128 : (dy * 3 + dx + 1) * 128],
                                mid1[:, 14 * c + dy : 14 * c + dy + 14, dx : dx + 28],
                                start=(idx == 0),
                                stop=(idx == 8),
                            )
                            idx += 1
                    nc.scalar.activation(
                        out2[:, c * NCH : (c + 1) * NCH], ps2[:], Relu, bias=bias2[:]
                    )
                w_l[s]["out2"] = out2

            def emit_conv3(s):
                xs = xs_l[s]
                out2 = w_l[s]["out2"]
                w3c = w_l[s]["w3"]
                ofull = actp.tile([128, 4 * P], F16, tag="ofull", name=f"of{s}")
                for m in range(4):
                    for c in range(2):
                        ps3 = psconvp.tile([128, 14, 28], F32, tag="convps")
                        nc.tensor.matmul(
                            ps3[:],
                            w3c[:, m * 128 : (m + 1) * 128],
                            out2[:, c * NCH : (c + 1) * NCH],
                            start=True,
                            stop=(c == 1 and s < BS - 1),
                        )
                        dst = ofull[:, m * P + c * NCH : m * P + (c + 1) * NCH]
                        if c == 1 and s < BS - 1:
                            # residual via DVE STT + relu on the idle Pool
                            # engine (no PE identity matmul for this chunk)
                            u = residp.tile([128, NCH], F16, tag="u")
                            nc.vector.scalar_tensor_tensor(
                                u[:],
                                xs[m][:, c * NCH : (c + 1) * NCH],
                                bias3[:, m : m + 1],
                                ps3[:].rearrange("p a b -> p (a b)"),
                                op0=ADD,
                                op1=ADD,
                            )
                            nc.gpsimd.tensor_scalar_max(dst, u[:], 0.0)
                        else:
                            # residual via PE identity matmul; drain on ACT
                            # (DVE tensor_scalar for the last sample's c=1
                            # chunks, so the teardown drains run in parallel)
                            nc.tensor.matmul(
                                ps3[:],
                                ident,
                                xs[m][:, c * NCH : (c + 1) * NCH],
                                start=False,
                                stop=True,
                            )
                            if c == 1:
                                nc.vector.tensor_scalar(
                                    dst,
                                    ps3[:].rearrange("p a b -> p (a b)"),
                                    bias3[:, m : m + 1],
                                    0.0,
                                    op0=ADD,
                                    op1=MAX,
                                )
                            else:
                                nc.scalar.activation(
                                    dst,
                                    ps3[:].rearrange("p a b -> p (a b)"),
                                    Relu,
                                    bias=bias3[:, m : m + 1],
                                )
                    nc.sync.dma_start(
                        out_d[s, m], ofull[:, m * P : (m + 1) * P]
                    )
                del w_l[s]

            # ============ schedule ==========================================
            # Prelude: samples 0 AND 1 thread their conv1/conv2 between bank
            # arrivals, so the pipeline exits the DMA-bound front two samples
            # deep.  Router 2's reduces are issued as soon as x2 lands (it
            # arrives just before b3R) and finished after conv3(1), keeping
            # both the reduce latency and the in-order queues off the PE's
            # critical path.
            emit_router(0)
            emit_cw1(0)
            emit_conv1(0)
            emit_router(1)
            emit_cw1(1)
            emit_cw2(0)
            emit_conv1(1)
            emit_cw2(1)
            emit_conv2(0)
            emit_conv2(1)
            emit_cw3(0)
            emit_cw3(1)
            emit_router(2)
            emit_conv3(0)
            emit_router(3)
            emit_cw1(2)
            emit_cw2(2)
            emit_conv3(1)
            emit_cw3(2)
            emit_router(4)
            # Steady slots: convs of s interleaved with the combines of s+1
            # (their drains overlap the conv matmuls); router of s+3 last —
            # the ~2.3us reduce+sigmoid latency then has a full slot of
            # slack before cw1(s+3) consumes R, instead of stalling the PE
            # at each slot boundary.
            for s in range(2, BS):
                emit_conv1(s)
                if s + 1 < BS:
                    emit_cw1(s + 1)
                    emit_cw2(s + 1)
                emit_conv2(s)
                if s + 1 < BS:
                    emit_cw3(s + 1)
                emit_conv3(s)
                if s + 3 < BS:
                    emit_router(s + 3)

    nc.compile()
    return nc


_NC_CACHE = None


def _get_program():
    global _NC_CACHE
    if _NC_CACHE is None:
        _NC_CACHE = build_program()
    return _NC_CACHE


def prepare_inputs(
    x, router_w, router_b, w1, w2, w3,
    g1, b1, m1, v1, g2, b2, m2, v2, g3, b3, m3, v3,
):
    """Host-side preprocessing -> per-core in_maps."""
    f = np.float32
    x = np.asarray(x, f)
    router_w = np.asarray(router_w, f)
    router_b = np.asarray(router_b, f)
    w1 = np.asarray(w1, f)
    w2 = np.asarray(w2, f)
    w3 = np.asarray(w3, f)

    s1 = np.asarray(g1, f) / np.sqrt(np.asarray(v1, f) + EPS)
    s2 = np.asarray(g2, f) / np.sqrt(np.asarray(v2, f) + EPS)
    s3 = np.asarray(g3, f) / np.sqrt(np.asarray(v3, f) + EPS)
    bb1 = np.asarray(b1, f) - np.asarray(m1, f) * s1
    bb2 = np.asarray(b2, f) - np.asarray(m2, f) * s2
    bb3 = np.asarray(b3, f) - np.asarray(m3, f) * s3

    # Combined-weight layouts (per expert), matching the device tiles:
    #  Wb1[e, p, it*128+o] = w1s[e, o, it*128+p]
    w1s = w1[:, :, :, 0, 0] * s1[None, :, None]            # [E, o=128, i=512]
    Wb1 = (
        w1s.reshape(E, 128, 4, 128).transpose(0, 3, 2, 1).reshape(E, 128, 512)
    )
    #  Wb2[e, ci, tap*128+o]
    w2s = w2 * s2[None, :, None, None, None]               # [E, o, ci, dy, dx]
    Wb2 = (
        w2s.transpose(0, 3, 4, 2, 1).reshape(E, 9, 128, 128)
        .transpose(0, 2, 1, 3).reshape(E, 128, 1152)
    )
    #  Wb3[e, ci, o]
    w3s = w3[:, :, :, 0, 0] * s3[None, :, None]            # [E, o=512, ci=128]
    Wb3 = w3s.transpose(0, 2, 1)                           # [E, 128, 512]

    def to_bankR(Wb, ncols):
        # [E, 128(i), C] -> [(e,j16), (tc32, jt, i)] with C = 16*ncols.
        # Banks are 8x-scaled and stored fp8e4m3: they only carry the
        # per-sample DELTA sum_e 16*(rw_e-0.5) * bank_e (the expert-mean
        # wbar is added in fp16 at drain time), so fp8 quantization error
        # lands on a ~2% correction term.  jt is the DoubleRow k-tile.
        return np.ascontiguousarray(
            (Wb * 8.0).reshape(E, 128, ncols // 2, 2, 16)
            .transpose(0, 4, 2, 3, 1)          # [e, j16, tc32, jt, i]
            .reshape(128, -1)
        ).astype(ml_dtypes.float8_e4m3fn)

    bank1 = to_bankR(Wb1, NC1)
    bank2 = to_bankR(Wb2, NC2)
    bank3 = to_bankR(Wb3, NC3)
    wbar = np.concatenate(
        [0.5 * Wb1.sum(axis=0), 0.5 * Wb2.sum(axis=0), 0.5 * Wb3.sum(axis=0)],
        axis=1,
    ).astype(np.float16)

    cc = np.zeros((128, 896), np.float16)
    # RW4[i, it*128 + (e*16+j)] = router_w[e, it*128+i] / P
    rwt4 = (router_w / float(P)).reshape(E, 4, 128)        # [e, it, i]
    arr = np.repeat(rwt4.transpose(1, 2, 0)[:, :, :, None], 16, axis=3)
    cc[:, 0:512] = arr.reshape(4, 128, 128).transpose(1, 0, 2).reshape(128, 512)
    # pat[(e,j16), (jt,j')] = delta_{j' == 16*jt + j16}
    pat = np.tile(
        np.eye(32, dtype=np.float16).reshape(2, 16, 32).transpose(1, 0, 2)
        .reshape(16, 64),
        (8, 1),
    )
    cc[:, 512:576] = 16.0 * pat
    cc[:, 576:704] = np.eye(128, dtype=np.float16)
    cc[0, 704:832] = np.repeat(router_b.astype(np.float16), 16)
    cc[:, 832:896] = -8.0 * pat

    bias1 = bb1.reshape(128, 1)
    bias2 = bb2.reshape(128, 1)
    bias3 = np.ascontiguousarray(bb3.reshape(4, 128).T)

    x16 = x.reshape(B, 4, 128, P).astype(np.float16)

    shared = {
        "bank1": bank1,
        "bank2": bank2,
        "bank3": bank3,
        "wbar": wbar,
        "cc": cc,
        "bias1": bias1,
        "bias2": bias2,
        "bias3": bias3,
    }
    in_maps = []
    for c in range(NCORES):
        m = dict(shared)
        m["x"] = np.ascontiguousarray(x16[c * BS : (c + 1) * BS])
        in_maps.append(m)
    return in_maps


def run(in_maps, trace=False, tmpdir=None):
    nc = _get_program()
    res = bass_utils.run_bass_kernel_spmd(
        nc, in_maps, core_ids=list(range(NCORES)), trace=trace, tmpdir=tmpdir
    )
    outs = [np.asarray(r["out"], np.float32) for r in res.results]
    full = np.concatenate(outs, axis=0).reshape(B, CIN, H, H)
    return full, res


def kernel(**inputs):
    in_maps = prepare_inputs(**inputs)
    full, _ = run(in_maps, trace=False)
    return full


# revision 96
# speedup vs baseline: 1.0017x; 1.0017x over previous
"""
Bass/Trainium2 kernel for nn_BottleneckShared (moe_routing).

Computation (per sample b):
    rw   = sigmoid(mean_hw(x) @ router_w.T + router_b)          # [E]
    Wk_b = sum_e rw[e] * wk[e]            (k = 1,2,3)           # per-sample conv kernels
    out  = relu(bn3(conv3(relu(bn2(conv2(relu(bn1(conv1(x)))))))) + x)

Sharding: data-parallel over batch. 64 samples -> 8 NeuronCores x 8 samples.
Expert banks / router / BN params are replicated to every core.

Device-side design notes (v3):
 - BN scales folded into the expert banks on the host; BN biases applied in
   the PSUM-drain of each conv.
 - Weight combine via a block-diagonal router matrix streamed against
   STATIONARY bank chunks: bank chunk tc is a [128, 128] tile whose partition
   (e, j in 16) holds bank_e[:, 16*tc + j]; the moving operand is
   R_s[(e,j), j'] = rw_se * delta_jj' ([128, 16]).  Each matmul produces 16
   combined-weight columns directly:
     out[i, j'] = sum_{e,j} bank_e[i, 16tc+j] * rw_se delta_jj'
                = sum_e rw_se bank_e[i, 16tc+j'].
   PE cost is 16 columns per chunk (2176 columns/sample) vs 512 per expert
   (17408/sample) for the scaled-identity formulation.
 - Router computed transposed: rwT[(e,j), s] = sigmoid(sum_i RW4[i,(e,j)] *
   pooled_s[i] + b[e]) so R_s is one tensor_scalar_mul of a [128, 16] mask by
   the per-partition scalar rwT.
 - conv1/conv3 are 1x1 convs = matmuls over the 784 pixels; conv2 (3x3,
   pad 1) = 9 accumulating matmuls with shifted access patterns over a
   zero-padded [128, 30, 30] tile.
 - Residual: identity matmul accumulates x into the conv3 PSUM; bias+relu
   applied in the drains.
 - Engine balance per sample: PE 7.8us (convs+combines+residual+router),
   DVE ~6.9us (2 reduces, combine drains, conv3 c=1 drains), ACT ~6.7us
   (2 reduces via accum_out, sigmoid, conv1/2 drains, conv3 c=0 drains),
   Pool (memset only; GPSIMD cannot touch PSUM and cannot reduce the free
   axis, so it cannot take real work).  Conv PSUM tiles stay small
   ([128, 14, 28], bufs=4) — deep buffering decouples the PE from drain
   latency; pairing banks into wider tiles was tried and made conv3
   drain-paced.
 - Emission is software-pipelined one sample ahead: slot s runs convs of s
   on the PE while emitting the combines of s+1 (drains overlap convs) and
   the router of s+2 at slot end (keeps reduces behind the drains in the
   in-order DVE/ACT queues).  DMA order: x0, cc, b1R, biases, b2 pieces,
   x1, b3R, then remaining x in sample order; outputs trail each sample.
"""

import sys

import ml_dtypes
import numpy as np

sys.path.insert(0, "/opt/trn_rl_repo")

import concourse.bacc as bacc
import concourse.bass as bass
import concourse.mybir as mybir
import concourse.tile as tile
from concourse import bass_utils

EPS = 1e-5

B = 64          # global batch
NCORES = 8
BS = B // NCORES  # samples per core
E = 8           # experts
CIN = 512
WID = 128       # bottleneck width
COUT = 512
H = 28
P = H * H       # 784 pixels
NCH = 392       # pixels per conv output chunk (14 rows)

NC1 = 32        # 16-col combine chunks for W1 (512 cols)
NC2 = 72        # for W2 (1152 cols)
NC3 = 32        # for W3 (512 cols)

F16 = mybir.dt.float16
F32 = mybir.dt.float32
F8 = mybir.dt.float8e4


def build_program():
    nc = bacc.Bacc("TRN2", target_bir_lowering=False, debug=False)

    # ---- DRAM I/O (per-core shapes) ----
    x_d = nc.dram_tensor("x", [BS, 4, 128, P], F16, kind="ExternalInput")
    b1_d = nc.dram_tensor("bank1", [128, NC1 // 2, 2, 128], F8, kind="ExternalInput")
    b2_d = nc.dram_tensor("bank2", [128, NC2 // 2, 2, 128], F8, kind="ExternalInput")
    b3_d = nc.dram_tensor("bank3", [128, NC3 // 2, 2, 128], F8, kind="ExternalInput")
    wbar_d = nc.dram_tensor("wbar", [128, 2176], F16, kind="ExternalInput")
    cc_d = nc.dram_tensor("cc", [128, 896], F16, kind="ExternalInput")
    bias1_d = nc.dram_tensor("bias1", [128, 1], F32, kind="ExternalInput")
    bias2_d = nc.dram_tensor("bias2", [128, 1], F32, kind="ExternalInput")
    bias3_d = nc.dram_tensor("bias3", [128, 4], F32, kind="ExternalInput")
    out_d = nc.dram_tensor("out", [BS, 4, 128, P], F16, kind="ExternalOutput")

    Relu = mybir.ActivationFunctionType.Relu
    Sigmoid = mybir.ActivationFunctionType.Sigmoid
    Copy = mybir.ActivationFunctionType.Copy
    ADD = mybir.AluOpType.add
    MAX = mybir.AluOpType.max

    with tile.TileContext(nc) as tc:
        with (
            tc.tile_pool(name="const", bufs=1) as constp,
            tc.tile_pool(name="xin", bufs=6) as xp,
            tc.tile_pool(name="xsplit", bufs=1) as xsp,
            tc.tile_pool(name="comb", bufs=2) as combp,
            tc.tile_pool(name="act", bufs=4) as actp,
            tc.tile_pool(name="small", bufs=4) as smallp,
            tc.tile_pool(name="rmat", bufs=6) as rp,
            tc.tile_pool(name="resid", bufs=4) as residp,
            tc.tile_pool(name="pscomb", bufs=2, space=bass.MemorySpace.PSUM) as pscombp,
            tc.tile_pool(name="psconv", bufs=5, space=bass.MemorySpace.PSUM) as psconvp,
            tc.tile_pool(name="psr", bufs=1, space=bass.MemorySpace.PSUM) as psrp,
        ):
            # ---- persistent constants ----
            # banks: [(e,j16), tc32, jt, i], 8x scaled fp8 (DoubleRow k-tiles)
            bank1 = constp.tile([128, NC1 // 2, 2, 128], F8)
            bank2 = constp.tile([128, NC2 // 2, 2, 128], F8)
            bank3 = constp.tile([128, NC3 // 2, 2, 128], F8)
            wbar = constp.tile([128, 2176], F16)       # 0.5*sum_e bank_e
            cc = constp.tile([128, 896], F16)
            rw4 = cc[:, 0:512]        # RW4[i, (it,(e,j))]
            mask32 = cc[:, 512:576]   # [(e,j16),(jt,j')] = 16*delta_{16jt+j16,j'}
            ident = cc[:, 576:704]    # I128 for the residual matmul
            rbT = cc[0:1, 704:832]    # row 0: router_b[(e,j)]
            maskneg = cc[:, 832:896]  # -8*delta pattern
            ones1 = constp.tile([1, 1], F16)
            bias1 = constp.tile([128, 1], F32)
            bias2 = constp.tile([128, 1], F32)
            bias3 = constp.tile([128, 4], F32)

            nc.gpsimd.memset(ones1[:], 1.0)

            # DMA issue order = device service order. Sample 0's tiles and
            # the banks go first (they gate sample 0's convs); later samples'
            # x tiles follow the banks in need-order.
            xs_l, r_l = [], []
            for s in (0, 1):
                xs_l.append([
                    xsp.tile([128, P], F16, tag=f"xs{s}_{t}", name=f"xs{s}_{t}")
                    for t in range(4)
                ])
            xs0 = xs_l[0]
            for s in range(2, BS):
                big = xp.tile([128, 4 * P], F16, tag="xs", name=f"xs{s}")
                xs_l.append([big[:, t * P : (t + 1) * P] for t in range(4)])
            nc.sync.dma_start(xs0[0][:], x_d[0, 0])
            nc.sync.dma_start(xs0[1][:], x_d[0, 1])
            nc.sync.dma_start(xs0[2][:], x_d[0, 2])
            nc.sync.dma_start(xs0[3][:], x_d[0, 3])
            nc.sync.dma_start(cc[:], cc_d[:])
            nc.sync.dma_start(xs_l[1][0][:], x_d[1, 0])
            nc.sync.dma_start(xs_l[1][1][:], x_d[1, 1])
            nc.sync.dma_start(xs_l[1][2][:], x_d[1, 2])
            nc.sync.dma_start(xs_l[1][3][:], x_d[1, 3])
            nc.sync.dma_start(bank1[:], b1_d[:])
            nc.sync.dma_start(wbar[:, 0:512], wbar_d[:, 0:512])
            nc.sync.dma_start(bias1[:], bias1_d[:])
            nc.sync.dma_start(bias2[:], bias2_d[:])
            nc.sync.dma_start(bias3[:], bias3_d[:])
            nc.sync.dma_start(bank2[:, 0:16], b2_d[:, 0:16])
            nc.sync.dma_start(wbar[:, 512:1664], wbar_d[:, 512:1664])
            nc.sync.dma_start(bank2[:, 16:32], b2_d[:, 16:32])
            nc.sync.dma_start(bank2[:, 32:36], b2_d[:, 32:36])
            nc.sync.dma_start(
                xs_l[2][0].tensor[:, :], x_d[2].transpose([1, 0, 2])
            )
            nc.sync.dma_start(bank3[:], b3_d[:])
            nc.sync.dma_start(wbar[:, 1664:2176], wbar_d[:, 1664:2176])
            for s in range(3, BS):
                nc.sync.dma_start(
                    xs_l[s][0].tensor[:, :], x_d[s].transpose([1, 0, 2])
                )

            # ================= routers =====================================
            pooled_l = {}

            def emit_router_reduce(s):
                xs = xs_l[s]

                # Pooling split across DVE and ACT (accum_out).  For the
                # front samples (tiles landing 558ns apart) 3 DVE / 1 ACT
                # minimizes the latency to R; in steady state 2/2 balances
                # engine load.
                dve_tiles = (0, 1, 3) if s < 2 else (0, 1)
                pooled = smallp.tile([128, 4], F32, tag="pooled")
                for t in range(4):
                    if t in dve_tiles:
                        nc.vector.tensor_reduce(
                            pooled[:, t : t + 1],
                            xs[t][:, :],
                            axis=mybir.AxisListType.X,
                            op=ADD,
                        )
                    else:
                        scratch = smallp.tile([128, P], F16, tag="scratch")
                        nc.scalar.activation(
                            scratch[:],
                            xs[t][:, :],
                            Copy,
                            accum_out=pooled[:, t : t + 1],
                        )
                pooled_l[s] = pooled

            def emit_router_finish(s):
                pooled = pooled_l.pop(s)
                pooled16 = smallp.tile([128, 4], F16, tag="pooled16")
                nc.vector.tensor_copy(pooled16[:], pooled[:])

                rpsum = psrp.tile([128, 1], F32, tag="rpsum")
                for t in range(4):
                    nc.tensor.matmul(
                        rpsum[:],
                        rw4[:, t * 128 : (t + 1) * 128],
                        pooled16[:, t : t + 1],
                        start=(t == 0),
                        stop=False,
                    )
                nc.tensor.matmul(
                    rpsum[:],
                    rbT,
                    ones1[:],
                    start=False,
                    stop=True,
                )
                rwT = smallp.tile([128, 1], F32, tag="rwT", name=f"rwT{s}")
                nc.scalar.activation(rwT[:], rpsum[:], Sigmoid)

                rs = rp.tile([128, 2, 32], F8, tag="rs", name=f"rs{s}")
                nc.vector.scalar_tensor_tensor(
                    rs[:].rearrange("p t j -> p (t j)"),
                    mask32, rwT[:, 0:1], maskneg,
                    op0=mybir.AluOpType.mult, op1=ADD,
                )
                r_l.append(rs)

            def emit_router(s):
                emit_router_reduce(s)
                emit_router_finish(s)

            # ============ combines (bank chunks stationary) =================
            w_l = {}

            def combine(s, bank, a0, ncols, dst, d0, woff):
                # ncols 32-col chunks; DoubleRow fp8 matmuls contract 256
                # rows ((e, j in 32)) at 0.5 cycles/row
                rs = r_l[s]
                psc = pscombp.tile([128, 512], F32, tag="psc", name="psc")
                for a in range(ncols):
                    nc.tensor.matmul(
                        psc[:, 32 * a : 32 * a + 32],
                        bank[:, a0 + a, :, :],
                        rs[:],
                        start=True,
                        stop=True,
                        perf_mode=mybir.MatmulPerfMode.DoubleRow,
                    )
                # dst = wbar + psc/128  (the drain applies the fp8 scale
                # normalization and adds the expert-mean weights)
                nc.vector.scalar_tensor_tensor(
                    dst[:, d0 : d0 + 32 * ncols],
                    psc[:, : 32 * ncols],
                    1.0 / 128.0,
                    wbar[:, woff + d0 : woff + d0 + 32 * ncols],
                    op0=mybir.AluOpType.mult,
                    op1=ADD,
                )

            def emit_cw1(s):
                w1c = combp.tile([128, 512], F16, tag="w1c", name=f"w1c{s}")
                combine(s, bank1, 0, 16, w1c, 0, 0)
                w_l.setdefault(s, {})["w1"] = w1c

            def emit_cw2(s):
                w2c = combp.tile([128, 1152], F16, tag="w2c", name=f"w2c{s}")
                combine(s, bank2, 0, 16, w2c, 0, 512)
                combine(s, bank2, 16, 16, w2c, 512, 512)
                combine(s, bank2, 32, 4, w2c, 1024, 512)
                w_l.setdefault(s, {})["w2"] = w2c

            def emit_cw3(s):
                w3c = combp.tile([128, 512], F16, tag="w3c", name=f"w3c{s}")
                combine(s, bank3, 0, 16, w3c, 0, 1664)
                w_l.setdefault(s, {})["w3"] = w3c

            # ============ convs =============================================
            def emit_conv1(s):
                xs = xs_l[s]
                w1c = w_l[s]["w1"]
                mid1 = actp.tile([128, 30, 30], F16, tag="mid1", name=f"mid1_{s}")
                # only the border needs zeroing (interior is overwritten by
                # the conv1 drain)
                nc.gpsimd.memset(mid1[:, 0, :], 0.0)
                nc.gpsimd.memset(mid1[:, 29, :], 0.0)
                nc.gpsimd.memset(mid1[:, 1:29, 0:1], 0.0)
                nc.gpsimd.memset(mid1[:, 1:29, 29:30], 0.0)
                for c in range(2):
                    ps1 = psconvp.tile([128, 14, 28], F32, tag="convps")
                    for k in range(4):
                        nc.tensor.matmul(
                            ps1[:],
                            w1c[:, k * 128 : (k + 1) * 128],
                            xs[k][:, c * NCH : (c + 1) * NCH],
                            start=(k == 0),
                            stop=(k == 3),
                        )
                    nc.scalar.activation(
                        mid1[:, 14 * c + 1 : 14 * c + 15, 1:29],
                        ps1[:],
                        Relu,
                        bias=bias1[:],
                    )
                w_l[s]["mid1"] = mid1

            def emit_conv2(s):
                mid1 = w_l[s]["mid1"]
                w2c = w_l[s]["w2"]
                out2 = actp.tile([128, P], F16, tag="out2", name=f"out2_{s}")
                for c in range(2):
                    ps2 = psconvp.tile([128, 14, 28], F32, tag="convps")
                    idx = 0
                    for dy in range(3):
                        for dx in range(3):
                            nc.tensor.matmul(
                                ps2[:],
                                w2c[:, (dy * 3 + dx) * 128 : (dy * 3 + dx + 1) * 128],
                                mid1[:, 14 * c + dy : 14 * c + dy + 14, dx : dx + 28],
                                start=(idx == 0),
                                stop=(idx == 8),
                            )
                            idx += 1
                    nc.scalar.activation(
                        out2[:, c * NCH : (c + 1) * NCH], ps2[:], Relu, bias=bias2[:]
                    )
                w_l[s]["out2"] = out2

            def emit_conv3(s):
                xs = xs_l[s]
                out2 = w_l[s]["out2"]
                w3c = w_l[s]["w3"]
                ofull = actp.tile([128, 4 * P], F16, tag="ofull", name=f"of{s}")
                for m in range(4):
                    for c in range(2):
                        ps3 = psconvp.tile([128, 14, 28], F32, tag="convps")
                        nc.tensor.matmul(
                            ps3[:],
                            w3c[:, m * 128 : (m + 1) * 128],
                            out2[:, c * NCH : (c + 1) * NCH],
                            start=True,
                            stop=(c == 1 and s < BS - 1),
                        )
                        dst = ofull[:, m * P + c * NCH : m * P + (c + 1) * NCH]
                        if c == 1 and s < BS - 1:
                            # residual via DVE STT + relu on the idle Pool
                            # engine (no PE identity matmul for this chunk)
                            u = residp.tile([128, NCH], F16, tag="u")
                            nc.vector.scalar_tensor_tensor(
                                u[:],
                                xs[m][:, c * NCH : (c + 1) * NCH],
                                bias3[:, m : m + 1],
                                ps3[:].rearrange("p a b -> p (a b)"),
                                op0=ADD,
                                op1=ADD,
                            )
                            nc.gpsimd.tensor_scalar_max(dst, u[:], 0.0)
                        else:
                            # residual via PE identity matmul; drain on ACT
                            # (DVE tensor_scalar for the last sample's c=1
                            # chunks, so the teardown drains run in parallel)
                            nc.tensor.matmul(
                                ps3[:],
                                ident,
                                xs[m][:, c * NCH : (c + 1) * NCH],
                                start=False,
                                stop=True,
                            )
                            if c == 1:
                                nc.vector.tensor_scalar(
                                    dst,
                                    ps3[:].rearrange("p a b -> p (a b)"),
                                    bias3[:, m : m + 1],
                                    0.0,
                                    op0=ADD,
                                    op1=MAX,
                                )
                            else:
                                nc.scalar.activation(
                                    dst,
                                    ps3[:].rearrange("p a b -> p (a b)"),
                                    Relu,
                                    bias=bias3[:, m : m + 1],
                                )
                    nc.sync.dma_start(
                        out_d[s, m], ofull[:, m * P : (m + 1) * P]
                    )
                del w_l[s]

            # ============ schedule ==========================================
            # Prelude: samples 0 AND 1 thread their conv1/conv2 between bank
            # arrivals, so the pipeline exits the DMA-bound front two samples
            # deep.  Router 2's reduces are issued as soon as x2 lands (it
            # arrives just before b3R) and finished after conv3(1), keeping
            # both the reduce latency and the in-order queues off the PE's
            # critical path.
            emit_router(0)
            emit_cw1(0)
            emit_conv1(0)
            emit_router(1)
            emit_cw1(1)
            emit_cw2(0)
            emit_conv1(1)
            emit_cw2(1)
            emit_conv2(0)
            emit_conv2(1)
            emit_cw3(0)
            emit_cw3(1)
            emit_router(2)
            emit_conv3(0)
            emit_router(3)
            emit_cw1(2)
            emit_cw2(2)
            emit_conv3(1)
            emit_cw3(2)
            emit_router(4)
            # Steady slots: convs of s interleaved with the combines of s+1
            # (their drains overlap the conv matmuls); router of s+3 last —
            # the ~2.3us reduce+sigmoid latency then has a full slot of
            # slack before cw1(s+3) consumes R, instead of stalling the PE
            # at each slot boundary.
            for s in range(2, BS):
                emit_conv1(s)
                if s + 1 < BS:
                    emit_cw1(s + 1)
                    emit_cw2(s + 1)
                emit_conv2(s)
                if s + 1 < BS:
                    emit_cw3(s + 1)
                emit_conv3(s)
                if s + 3 < BS:
                    emit_router(s + 3)

    nc.compile()
    return nc


_NC_CACHE = None


def _get_program():
    global _NC_CACHE
    if _NC_CACHE is None:
        _NC_CACHE = build_program()
    return _NC_CACHE


def prepare_inputs(
    x, router_w, router_b, w1, w2, w3,
    g1, b1, m1, v1, g2, b2, m2, v2, g3, b3, m3, v3,
):
    """Host-side preprocessing -> per-core in_maps."""
    f = np.float32
    x = np.asarray(x, f)
    router_w = np.asarray(router_w, f)
    router_b = np.asarray(router_b, f)
    w1 = np.asarray(w1, f)
    w2 = np.asarray(w2, f)
    w3 = np.asarray(w3, f)

    s1 = np.asarray(g1, f) / np.sqrt(np.asarray(v1, f) + EPS)
    s2 = np.asarray(g2, f) / np.sqrt(np.asarray(v2, f) + EPS)
    s3 = np.asarray(g3, f) / np.sqrt(np.asarray(v3, f) + EPS)
    bb1 = np.asarray(b1, f) - np.asarray(m1, f) * s1
    bb2 = np.asarray(b2, f) - np.asarray(m2, f) * s2
    bb3 = np.asarray(b3, f) - np.asarray(m3, f) * s3

    # Combined-weight layouts (per expert), matching the device tiles:
    #  Wb1[e, p, it*128+o] = w1s[e, o, it*128+p]
    w1s = w1[:, :, :, 0, 0] * s1[None, :, None]            # [E, o=128, i=512]
    Wb1 = (
        w1s.reshape(E, 128, 4, 128).transpose(0, 3, 2, 1).reshape(E, 128, 512)
    )
    #  Wb2[e, ci, tap*128+o]
    w2s = w2 * s2[None, :, None, None, None]               # [E, o, ci, dy, dx]
    Wb2 = (
        w2s.transpose(0, 3, 4, 2, 1).reshape(E, 9, 128, 128)
        .transpose(0, 2, 1, 3).reshape(E, 128, 1152)
    )
    #  Wb3[e, ci, o]
    w3s = w3[:, :, :, 0, 0] * s3[None, :, None]            # [E, o=512, ci=128]
    Wb3 = w3s.transpose(0, 2, 1)                           # [E, 128, 512]

    def to_bankR(Wb, ncols):
        # [E, 128(i), C] -> [(e,j16), (tc32, jt, i)] with C = 16*ncols.
        # Banks are 8x-scaled and stored fp8e4m3: they only carry the
        # per-sample DELTA sum_e 16*(rw_e-0.5) * bank_e (the expert-mean
        # wbar is added in fp16 at drain time), so fp8 quantization error
        # lands on a ~2% correction term.  jt is the DoubleRow k-tile.
        return np.ascontiguousarray(
            (Wb * 8.0).reshape(E, 128, ncols // 2, 2, 16)
            .transpose(0, 4, 2, 3, 1)          # [e, j16, tc32, jt, i]
            .reshape(128, -1)
        ).astype(ml_dtypes.float8_e4m3fn)

    bank1 = to_bankR(Wb1, NC1)
    bank2 = to_bankR(Wb2, NC2)
    bank3 = to_bankR(Wb3, NC3)
    wbar = np.concatenate(
        [0.5 * Wb1.sum(axis=0), 0.5 * Wb2.sum(axis=0), 0.5 * Wb3.sum(axis=0)],
        axis=1,
    ).astype(np.float16)

    cc = np.zeros((128, 896), np.float16)
    # RW4[i, it*128 + (e*16+j)] = router_w[e, it*128+i] / P
    rwt4 = (router_w / float(P)).reshape(E, 4, 128)        # [e, it, i]
    arr = np.repeat(rwt4.transpose(1, 2, 0)[:, :, :, None], 16, axis=3)
    cc[:, 0:512] = arr.reshape(4, 128, 128).transpose(1, 0, 2).reshape(128, 512)
    # pat[(e,j16), (jt,j')] = delta_{j' == 16*jt + j16}
    pat = np.tile(
        np.eye(32, dtype=np.float16).reshape(2, 16, 32).transpose(1, 0, 2)
        .reshape(16, 64),
        (8, 1),
    )
    cc[:, 512:576] = 16.0 * pat
    cc[:, 576:704] = np.eye(128, dtype=np.float16)
    cc[0, 704:832] = np.repeat(router_b.astype(np.float16), 16)
    cc[:, 832:896] = -8.0 * pat

    bias1 = bb1.reshape(128, 1)
    bias2 = bb2.reshape(128, 1)
    bias3 = np.ascontiguousarray(bb3.reshape(4, 128).T)

    x16 = x.reshape(B, 4, 128, P).astype(np.float16)

    shared = {
        "bank1": bank1,
        "bank2": bank2,
        "bank3": bank3,
        "wbar": wbar,
        "cc": cc,
        "bias1": bias1,
        "bias2": bias2,
        "bias3": bias3,
    }
    in_maps = []
    for c in range(NCORES):
        m = dict(shared)
        m["x"] = np.ascontiguousarray(x16[c * BS : (c + 1) * BS])
        in_maps.append(m)
    return in_maps


def run(in_maps, trace=False, tmpdir=None):
    nc = _get_program()
    res = bass_utils.run_bass_kernel_spmd(
        nc, in_maps, core_ids=list(range(NCORES)), trace=trace, tmpdir=tmpdir
    )
    outs = [np.asarray(r["out"], np.float32) for r in res.results]
    full = np.concatenate(outs, axis=0).reshape(B, CIN, H, H)
    return full, res


def kernel(**inputs):
    in_maps = prepare_inputs(**inputs)
    full, _ = run(in_maps, trace=False)
    return full


# revision 99
# speedup vs baseline: 1.0170x; 1.0153x over previous
"""
Bass/Trainium2 kernel for nn_BottleneckShared (moe_routing).

Computation (per sample b):
    rw   = sigmoid(mean_hw(x) @ router_w.T + router_b)          # [E]
    Wk_b = sum_e rw[e] * wk[e]            (k = 1,2,3)           # per-sample conv kernels
    out  = relu(bn3(conv3(relu(bn2(conv2(relu(bn1(conv1(x)))))))) + x)

Sharding: data-parallel over batch. 64 samples -> 8 NeuronCores x 8 samples.
Expert banks / router / BN params are replicated to every core.

Device-side design notes (v3):
 - BN scales folded into the expert banks on the host; BN biases applied in
   the PSUM-drain of each conv.
 - Weight combine via a block-diagonal router matrix streamed against
   STATIONARY bank chunks: bank chunk tc is a [128, 128] tile whose partition
   (e, j in 16) holds bank_e[:, 16*tc + j]; the moving operand is
   R_s[(e,j), j'] = rw_se * delta_jj' ([128, 16]).  Each matmul produces 16
   combined-weight columns directly:
     out[i, j'] = sum_{e,j} bank_e[i, 16tc+j] * rw_se delta_jj'
                = sum_e rw_se bank_e[i, 16tc+j'].
   PE cost is 16 columns per chunk (2176 columns/sample) vs 512 per expert
   (17408/sample) for the scaled-identity formulation.
 - Router computed transposed: rwT[(e,j), s] = sigmoid(sum_i RW4[i,(e,j)] *
   pooled_s[i] + b[e]) so R_s is one tensor_scalar_mul of a [128, 16] mask by
   the per-partition scalar rwT.
 - conv1/conv3 are 1x1 convs = matmuls over the 784 pixels; conv2 (3x3,
   pad 1) = 9 accumulating matmuls with shifted access patterns over a
   zero-padded [128, 30, 30] tile.
 - Residual: identity matmul accumulates x into the conv3 PSUM; bias+relu
   applied in the drains.
 - Engine balance per sample: PE 7.8us (convs+combines+residual+router),
   DVE ~6.9us (2 reduces, combine drains, conv3 c=1 drains), ACT ~6.7us
   (2 reduces via accum_out, sigmoid, conv1/2 drains, conv3 c=0 drains),
   Pool (memset only; GPSIMD cannot touch PSUM and cannot reduce the free
   axis, so it cannot take real work).  Conv PSUM tiles stay small
   ([128, 14, 28], bufs=4) — deep buffering decouples the PE from drain
   latency; pairing banks into wider tiles was tried and made conv3
   drain-paced.
 - Emission is software-pipelined one sample ahead: slot s runs convs of s
   on the PE while emitting the combines of s+1 (drains overlap convs) and
   the router of s+2 at slot end (keeps reduces behind the drains in the
   in-order DVE/ACT queues).  DMA order: x0, cc, b1R, biases, b2 pieces,
   x1, b3R, then remaining x in sample order; outputs trail each sample.
"""

import sys

import ml_dtypes
import numpy as np

sys.path.insert(0, "/opt/trn_rl_repo")

import concourse.bacc as bacc
import concourse.bass as bass
import concourse.mybir as mybir
import concourse.tile as tile
from concourse import bass_utils

EPS = 1e-5

B = 64          # global batch
NCORES = 8
BS = B // NCORES  # samples per core
E = 8           # experts
CIN = 512
WID = 128       # bottleneck width
COUT = 512
H = 28
P = H * H       # 784 pixels
NCH = 392       # pixels per conv output chunk (14 rows)

NC1 = 32        # 16-col combine chunks for W1 (512 cols)
NC2 = 72        # for W2 (1152 cols)
NC3 = 32        # for W3 (512 cols)

F16 = mybir.dt.float16
F32 = mybir.dt.float32
F8 = mybir.dt.float8e4


def build_program():
    nc = bacc.Bacc("TRN2", target_bir_lowering=False, debug=False)

    # ---- DRAM I/O (per-core shapes) ----
    x_d = nc.dram_tensor("x", [BS, 4, 128, P], F16, kind="ExternalInput")
    b1_d = nc.dram_tensor("bank1", [128, NC1 // 2, 2, 128], F8, kind="ExternalInput")
    b2_d = nc.dram_tensor("bank2", [128, NC2 // 2, 2, 128], F8, kind="ExternalInput")
    b3_d = nc.dram_tensor("bank3", [128, NC3 // 2, 2, 128], F8, kind="ExternalInput")
    wbar_d = nc.dram_tensor("wbar", [128, 2176], F16, kind="ExternalInput")
    cc_d = nc.dram_tensor("cc", [128, 896], F16, kind="ExternalInput")
    biases_d = nc.dram_tensor("biases", [128, 6], F32, kind="ExternalInput")
    out_d = nc.dram_tensor("out", [BS, 4, 128, P], F16, kind="ExternalOutput")

    Relu = mybir.ActivationFunctionType.Relu
    Sigmoid = mybir.ActivationFunctionType.Sigmoid
    Copy = mybir.ActivationFunctionType.Copy
    ADD = mybir.AluOpType.add
    MAX = mybir.AluOpType.max

    with tile.TileContext(nc) as tc:
        with (
            tc.tile_pool(name="const", bufs=1) as constp,
            tc.tile_pool(name="xin", bufs=6) as xp,
            tc.tile_pool(name="xsplit", bufs=1) as xsp,
            tc.tile_pool(name="comb", bufs=2) as combp,
            tc.tile_pool(name="act", bufs=4) as actp,
            tc.tile_pool(name="small", bufs=4) as smallp,
            tc.tile_pool(name="rmat", bufs=6) as rp,
            tc.tile_pool(name="resid", bufs=4) as residp,
            tc.tile_pool(name="pscomb", bufs=2, space=bass.MemorySpace.PSUM) as pscombp,
            tc.tile_pool(name="psconv", bufs=5, space=bass.MemorySpace.PSUM) as psconvp,
            tc.tile_pool(name="psr", bufs=1, space=bass.MemorySpace.PSUM) as psrp,
        ):
            # ---- persistent constants ----
            # banks: [(e,j16), tc32, jt, i], 8x scaled fp8 (DoubleRow k-tiles)
            bank1 = constp.tile([128, NC1 // 2, 2, 128], F8)
            bank2 = constp.tile([128, NC2 // 2, 2, 128], F8)
            bank3 = constp.tile([128, NC3 // 2, 2, 128], F8)
            wbar = constp.tile([128, 2176], F16)       # 0.5*sum_e bank_e
            cc = constp.tile([128, 896], F16)
            rw4 = cc[:, 0:512]        # RW4[i, (it,(e,j))]
            mask32 = cc[:, 512:576]   # [(e,j16),(jt,j')] = 16*delta_{16jt+j16,j'}
            ident = cc[:, 576:704]    # I128 for the residual matmul
            rbT = cc[0:1, 704:832]    # row 0: router_b[(e,j)]
            maskneg = cc[:, 832:896]  # -8*delta pattern
            ones1 = constp.tile([1, 1], F16)
            biases = constp.tile([128, 6], F32)
            bias1 = biases[:, 0:1]
            bias2 = biases[:, 1:2]
            bias3 = biases[:, 2:6]

            nc.gpsimd.memset(ones1[:], 1.0)

            # DMA issue order = device service order. Sample 0's tiles and
            # the banks go first (they gate sample 0's convs); later samples'
            # x tiles follow the banks in need-order.
            xs_l, r_l = [], []
            for s in (0, 1):
                xs_l.append([
                    xsp.tile([128, P], F16, tag=f"xs{s}_{t}", name=f"xs{s}_{t}")
                    for t in range(4)
                ])
            xs0 = xs_l[0]
            for s in range(2, BS):
                big = xp.tile([128, 4 * P], F16, tag="xs", name=f"xs{s}")
                xs_l.append([big[:, t * P : (t + 1) * P] for t in range(4)])
            nc.sync.dma_start(xs0[0][:], x_d[0, 0])
            nc.sync.dma_start(xs0[1][:], x_d[0, 1])
            nc.sync.dma_start(xs0[2][:], x_d[0, 2])
            nc.sync.dma_start(xs0[3][:], x_d[0, 3])
            nc.sync.dma_start(cc[:], cc_d[:])
            nc.sync.dma_start(xs_l[1][0][:], x_d[1, 0])
            nc.sync.dma_start(xs_l[1][1][:], x_d[1, 1])
            nc.sync.dma_start(xs_l[1][2][:], x_d[1, 2])
            nc.sync.dma_start(xs_l[1][3][:], x_d[1, 3])
            nc.sync.dma_start(bank1[:], b1_d[:])
            nc.sync.dma_start(wbar[:, 0:512], wbar_d[:, 0:512])
            nc.sync.dma_start(biases[:], biases_d[:])
            nc.sync.dma_start(bank2[:, 0:16], b2_d[:, 0:16])
            nc.sync.dma_start(wbar[:, 512:1664], wbar_d[:, 512:1664])
            nc.sync.dma_start(bank2[:, 16:32], b2_d[:, 16:32])
            nc.sync.dma_start(bank2[:, 32:36], b2_d[:, 32:36])
            nc.sync.dma_start(
                xs_l[2][0].tensor[:, :], x_d[2].transpose([1, 0, 2])
            )
            nc.sync.dma_start(bank3[:], b3_d[:])
            nc.sync.dma_start(wbar[:, 1664:2176], wbar_d[:, 1664:2176])
            for s in range(3, BS):
                nc.sync.dma_start(
                    xs_l[s][0].tensor[:, :], x_d[s].transpose([1, 0, 2])
                )

            # ================= routers =====================================
            pooled_l = {}

            def emit_router_reduce(s):
                xs = xs_l[s]

                # Pooling split across DVE and ACT (accum_out).  For the
                # front samples (tiles landing 558ns apart) 3 DVE / 1 ACT
                # minimizes the latency to R; in steady state 2/2 balances
                # engine load.
                dve_tiles = (0, 1, 3) if s < 2 else (0, 1)
                pooled = smallp.tile([128, 4], F32, tag="pooled")
                for t in range(4):
                    if t in dve_tiles:
                        nc.vector.tensor_reduce(
                            pooled[:, t : t + 1],
                            xs[t][:, :],
                            axis=mybir.AxisListType.X,
                            op=ADD,
                        )
                    else:
                        scratch = smallp.tile([128, P], F16, tag="scratch")
                        nc.scalar.activation(
                            scratch[:],
                            xs[t][:, :],
                            Copy,
                            accum_out=pooled[:, t : t + 1],
                        )
                pooled_l[s] = pooled

            def emit_router_finish(s):
                pooled = pooled_l.pop(s)
                pooled16 = smallp.tile([128, 4], F16, tag="pooled16")
                nc.vector.tensor_copy(pooled16[:], pooled[:])

                rpsum = psrp.tile([128, 1], F32, tag="rpsum")
                for t in range(4):
                    nc.tensor.matmul(
                        rpsum[:],
                        rw4[:, t * 128 : (t + 1) * 128],
                        pooled16[:, t : t + 1],
                        start=(t == 0),
                        stop=False,
                    )
                nc.tensor.matmul(
                    rpsum[:],
                    rbT,
                    ones1[:],
                    start=False,
                    stop=True,
                )
                rwT = smallp.tile([128, 1], F32, tag="rwT", name=f"rwT{s}")
                nc.scalar.activation(rwT[:], rpsum[:], Sigmoid)

                rs = rp.tile([128, 2, 32], F8, tag="rs", name=f"rs{s}")
                nc.vector.scalar_tensor_tensor(
                    rs[:].rearrange("p t j -> p (t j)"),
                    mask32, rwT[:, 0:1], maskneg,
                    op0=mybir.AluOpType.mult, op1=ADD,
                )
                r_l.append(rs)

            def emit_router(s):
                emit_router_reduce(s)
                emit_router_finish(s)

            # ============ combines (bank chunks stationary) =================
            w_l = {}

            def combine(s, bank, a0, ncols, dst, d0, woff):
                # ncols 32-col chunks; DoubleRow fp8 matmuls contract 256
                # rows ((e, j in 32)) at 0.5 cycles/row
                rs = r_l[s]
                psc = pscombp.tile([128, 512], F32, tag="psc", name="psc")
                for a in range(ncols):
                    nc.tensor.matmul(
                        psc[:, 32 * a : 32 * a + 32],
                        bank[:, a0 + a, :, :],
                        rs[:],
                        start=True,
                        stop=True,
                        perf_mode=mybir.MatmulPerfMode.DoubleRow,
                    )
                # dst = wbar + psc/128  (the drain applies the fp8 scale
                # normalization and adds the expert-mean weights)
                nc.vector.scalar_tensor_tensor(
                    dst[:, d0 : d0 + 32 * ncols],
                    psc[:, : 32 * ncols],
                    1.0 / 128.0,
                    wbar[:, woff + d0 : woff + d0 + 32 * ncols],
                    op0=mybir.AluOpType.mult,
                    op1=ADD,
                )

            def emit_cw1(s):
                w1c = combp.tile([128, 512], F16, tag="w1c", name=f"w1c{s}")
                combine(s, bank1, 0, 16, w1c, 0, 0)
                w_l.setdefault(s, {})["w1"] = w1c

            def emit_cw2(s):
                w2c = combp.tile([128, 1152], F16, tag="w2c", name=f"w2c{s}")
                combine(s, bank2, 0, 16, w2c, 0, 512)
                combine(s, bank2, 16, 16, w2c, 512, 512)
                combine(s, bank2, 32, 4, w2c, 1024, 512)
                w_l.setdefault(s, {})["w2"] = w2c

            def emit_cw3(s):
                w3c = combp.tile([128, 512], F16, tag="w3c", name=f"w3c{s}")
                combine(s, bank3, 0, 16, w3c, 0, 1664)
                w_l.setdefault(s, {})["w3"] = w3c

            # ============ convs =============================================
            def emit_conv1(s):
                xs = xs_l[s]
                w1c = w_l[s]["w1"]
                mid1 = actp.tile([128, 30, 30], F16, tag="mid1", name=f"mid1_{s}")
                # only the border needs zeroing (interior is overwritten by
                # the conv1 drain)
                nc.gpsimd.memset(mid1[:, 0, :], 0.0)
                nc.gpsimd.memset(mid1[:, 29, :], 0.0)
                nc.gpsimd.memset(mid1[:, 1:29, 0:1], 0.0)
                nc.gpsimd.memset(mid1[:, 1:29, 29:30], 0.0)
                for c in range(2):
                    ps1 = psconvp.tile([128, 14, 28], F32, tag="convps")
                    for k in range(4):
                        nc.tensor.matmul(
                            ps1[:],
                            w1c[:, k * 128 : (k + 1) * 128],
                            xs[k][:, c * NCH : (c + 1) * NCH],
                            start=(k == 0),
                            stop=(k == 3),
                        )
                    nc.scalar.activation(
                        mid1[:, 14 * c + 1 : 14 * c + 15, 1:29],
                        ps1[:],
                        Relu,
                        bias=bias1,
                    )
                w_l[s]["mid1"] = mid1

            def emit_conv2(s):
                mid1 = w_l[s]["mid1"]
                w2c = w_l[s]["w2"]
                out2 = actp.tile([128, P], F16, tag="out2", name=f"out2_{s}")
                for c in range(2):
                    ps2 = psconvp.tile([128, 14, 28], F32, tag="convps")
                    idx = 0
                    for dy in range(3):
                        for dx in range(3):
                            nc.tensor.matmul(
                                ps2[:],
                                w2c[:, (dy * 3 + dx) * 128 : (dy * 3 + dx + 1) * 128],
                                mid1[:, 14 * c + dy : 14 * c + dy + 14, dx : dx + 28],
                                start=(idx == 0),
                                stop=(idx == 8),
                            )
                            idx += 1
                    nc.scalar.activation(
                        out2[:, c * NCH : (c + 1) * NCH], ps2[:], Relu, bias=bias2
                    )
                w_l[s]["out2"] = out2

            def emit_conv3(s):
                xs = xs_l[s]
                out2 = w_l[s]["out2"]
                w3c = w_l[s]["w3"]
                ofull = actp.tile([128, 4 * P], F16, tag="ofull", name=f"of{s}")
                for m in range(4):
                    for c in range(2):
                        ps3 = psconvp.tile([128, 14, 28], F32, tag="convps")
                        nc.tensor.matmul(
                            ps3[:],
                            w3c[:, m * 128 : (m + 1) * 128],
                            out2[:, c * NCH : (c + 1) * NCH],
                            start=True,
                            stop=(c == 1 and s < BS - 1),
                        )
                        dst = ofull[:, m * P + c * NCH : m * P + (c + 1) * NCH]
                        if c == 1 and s < BS - 1:
                            # residual via DVE STT + relu on the idle Pool
                            # engine (no PE identity matmul for this chunk)
                            u = residp.tile([128, NCH], F16, tag="u")
                            nc.vector.scalar_tensor_tensor(
                                u[:],
                                xs[m][:, c * NCH : (c + 1) * NCH],
                                bias3[:, m : m + 1],
                                ps3[:].rearrange("p a b -> p (a b)"),
                                op0=ADD,
                                op1=ADD,
                            )
                            nc.gpsimd.tensor_scalar_max(dst, u[:], 0.0)
                        else:
                            # residual via PE identity matmul; drain on ACT
                            # (DVE tensor_scalar for the last sample's c=1
                            # chunks, so the teardown drains run in parallel)
                            nc.tensor.matmul(
                                ps3[:],
                                ident,
                                xs[m][:, c * NCH : (c + 1) * NCH],
                                start=False,
                                stop=True,
                            )
                            if c == 1:
                                nc.vector.tensor_scalar(
                                    dst,
                                    ps3[:].rearrange("p a b -> p (a b)"),
                                    bias3[:, m : m + 1],
                                    0.0,
                                    op0=ADD,
                                    op1=MAX,
                                )
                            else:
                                nc.scalar.activation(
                                    dst,
                                    ps3[:].rearrange("p a b -> p (a b)"),
                                    Relu,
                                    bias=bias3[:, m : m + 1],
                                )
                    nc.sync.dma_start(
                        out_d[s, m], ofull[:, m * P : (m + 1) * P]
                    )
                del w_l[s]

            # ============ schedule ==========================================
            # Prelude: samples 0 AND 1 thread their conv1/conv2 between bank
            # arrivals, so the pipeline exits the DMA-bound front two samples
            # deep.  Router 2's reduces are issued as soon as x2 lands (it
            # arrives just before b3R) and finished after conv3(1), keeping
            # both the reduce latency and the in-order queues off the PE's
            # critical path.
            emit_router(0)
            emit_cw1(0)
            emit_conv1(0)
            emit_router(1)
            emit_cw1(1)
            emit_cw2(0)
            emit_conv1(1)
            emit_cw2(1)
            emit_conv2(0)
            emit_conv2(1)
            emit_cw3(0)
            emit_cw3(1)
            emit_router(2)
            emit_conv3(0)
            emit_router(3)
            emit_cw1(2)
            emit_cw2(2)
            emit_conv3(1)
            emit_cw3(2)
            emit_router(4)
            # Steady slots: convs of s interleaved with the combines of s+1
            # (their drains overlap the conv matmuls); router of s+3 last —
            # the ~2.3us reduce+sigmoid latency then has a full slot of
            # slack before cw1(s+3) consumes R, instead of stalling the PE
            # at each slot boundary.
            for s in range(2, BS):
                emit_conv1(s)
                if s + 1 < BS:
                    emit_cw1(s + 1)
                    emit_cw2(s + 1)
                emit_conv2(s)
                if s + 1 < BS:
                    emit_cw3(s + 1)
                emit_conv3(s)
                if s + 3 < BS:
                    emit_router(s + 3)

    nc.compile()
    return nc


_NC_CACHE = None


def _get_program():
    global _NC_CACHE
    if _NC_CACHE is None:
        _NC_CACHE = build_program()
    return _NC_CACHE


def prepare_inputs(
    x, router_w, router_b, w1, w2, w3,
    g1, b1, m1, v1, g2, b2, m2, v2, g3, b3, m3, v3,
):
    """Host-side preprocessing -> per-core in_maps."""
    f = np.float32
    x = np.asarray(x, f)
    router_w = np.asarray(router_w, f)
    router_b = np.asarray(router_b, f)
    w1 = np.asarray(w1, f)
    w2 = np.asarray(w2, f)
    w3 = np.asarray(w3, f)

    s1 = np.asarray(g1, f) / np.sqrt(np.asarray(v1, f) + EPS)
    s2 = np.asarray(g2, f) / np.sqrt(np.asarray(v2, f) + EPS)
    s3 = np.asarray(g3, f) / np.sqrt(np.asarray(v3, f) + EPS)
    bb1 = np.asarray(b1, f) - np.asarray(m1, f) * s1
    bb2 = np.asarray(b2, f) - np.asarray(m2, f) * s2
    bb3 = np.asarray(b3, f) - np.asarray(m3, f) * s3

    # Combined-weight layouts (per expert), matching the device tiles:
    #  Wb1[e, p, it*128+o] = w1s[e, o, it*128+p]
    w1s = w1[:, :, :, 0, 0] * s1[None, :, None]            # [E, o=128, i=512]
    Wb1 = (
        w1s.reshape(E, 128, 4, 128).transpose(0, 3, 2, 1).reshape(E, 128, 512)
    )
    #  Wb2[e, ci, tap*128+o]
    w2s = w2 * s2[None, :, None, None, None]               # [E, o, ci, dy, dx]
    Wb2 = (
        w2s.transpose(0, 3, 4, 2, 1).reshape(E, 9, 128, 128)
        .transpose(0, 2, 1, 3).reshape(E, 128, 1152)
    )
    #  Wb3[e, ci, o]
    w3s = w3[:, :, :, 0, 0] * s3[None, :, None]            # [E, o=512, ci=128]
    Wb3 = w3s.transpose(0, 2, 1)                           # [E, 128, 512]

    def to_bankR(Wb, ncols):
        # [E, 128(i), C] -> [(e,j16), (tc32, jt, i)] with C = 16*ncols.
        # Banks are 8x-scaled and stored fp8e4m3: they only carry the
        # per-sample DELTA sum_e 16*(rw_e-0.5) * bank_e (the expert-mean
        # wbar is added in fp16 at drain time), so fp8 quantization error
        # lands on a ~2% correction term.  jt is the DoubleRow k-tile.
        return np.ascontiguousarray(
            (Wb * 8.0).reshape(E, 128, ncols // 2, 2, 16)
            .transpose(0, 4, 2, 3, 1)          # [e, j16, tc32, jt, i]
            .reshape(128, -1)
        ).astype(ml_dtypes.float8_e4m3fn)

    bank1 = to_bankR(Wb1, NC1)
    bank2 = to_bankR(Wb2, NC2)
    bank3 = to_bankR(Wb3, NC3)
    wbar = np.concatenate(
        [0.5 * Wb1.sum(axis=0), 0.5 * Wb2.sum(axis=0), 0.5 * Wb3.sum(axis=0)],
        axis=1,
    ).astype(np.float16)

    cc = np.zeros((128, 896), np.float16)
    # RW4[i, it*128 + (e*16+j)] = router_w[e, it*128+i] / P
    rwt4 = (router_w / float(P)).reshape(E, 4, 128)        # [e, it, i]
    arr = np.repeat(rwt4.transpose(1, 2, 0)[:, :, :, None], 16, axis=3)
    cc[:, 0:512] = arr.reshape(4, 128, 128).transpose(1, 0, 2).reshape(128, 512)
    # pat[(e,j16), (jt,j')] = delta_{j' == 16*jt + j16}
    pat = np.tile(
        np.eye(32, dtype=np.float16).reshape(2, 16, 32).transpose(1, 0, 2)
        .reshape(16, 64),
        (8, 1),
    )
    cc[:, 512:576] = 16.0 * pat
    cc[:, 576:704] = np.eye(128, dtype=np.float16)
    cc[0, 704:832] = np.repeat(router_b.astype(np.float16), 16)
    cc[:, 832:896] = -8.0 * pat

    biases = np.concatenate(
        [bb1.reshape(128, 1), bb2.reshape(128, 1), bb3.reshape(4, 128).T],
        axis=1,
    ).astype(np.float32)

    x16 = x.reshape(B, 4, 128, P).astype(np.float16)

    shared = {
        "bank1": bank1,
        "bank2": bank2,
        "bank3": bank3,
        "wbar": wbar,
        "cc": cc,
        "biases": biases,
    }
    in_maps = []
    for c in range(NCORES):
        m = dict(shared)
        m["x"] = np.ascontiguousarray(x16[c * BS : (c + 1) * BS])
        in_maps.append(m)
    return in_maps


def run(in_maps, trace=False, tmpdir=None):
    nc = _get_program()
    res = bass_utils.run_bass_kernel_spmd(
        nc, in_maps, core_ids=list(range(NCORES)), trace=trace, tmpdir=tmpdir
    )
    outs = [np.asarray(r["out"], np.float32) for r in res.results]
    full = np.concatenate(outs, axis=0).reshape(B, CIN, H, H)
    return full, res


def kernel(**inputs):
    in_maps = prepare_inputs(**inputs)
    full, _ = run(in_maps, trace=False)
    return full


# revision 102
# speedup vs baseline: 1.0197x; 1.0026x over previous
"""
Bass/Trainium2 kernel for nn_BottleneckShared (moe_routing).

Computation (per sample b):
    rw   = sigmoid(mean_hw(x) @ router_w.T + router_b)          # [E]
    Wk_b = sum_e rw[e] * wk[e]            (k = 1,2,3)           # per-sample conv kernels
    out  = relu(bn3(conv3(relu(bn2(conv2(relu(bn1(conv1(x)))))))) + x)

Sharding: data-parallel over batch. 64 samples -> 8 NeuronCores x 8 samples.
Expert banks / router / BN params are replicated to every core.

Device-side design notes (v3):
 - BN scales folded into the expert banks on the host; BN biases applied in
   the PSUM-drain of each conv.
 - Weight combine via a block-diagonal router matrix streamed against
   STATIONARY bank chunks: bank chunk tc is a [128, 128] tile whose partition
   (e, j in 16) holds bank_e[:, 16*tc + j]; the moving operand is
   R_s[(e,j), j'] = rw_se * delta_jj' ([128, 16]).  Each matmul produces 16
   combined-weight columns directly:
     out[i, j'] = sum_{e,j} bank_e[i, 16tc+j] * rw_se delta_jj'
                = sum_e rw_se bank_e[i, 16tc+j'].
   PE cost is 16 columns per chunk (2176 columns/sample) vs 512 per expert
   (17408/sample) for the scaled-identity formulation.
 - Router computed transposed: rwT[(e,j), s] = sigmoid(sum_i RW4[i,(e,j)] *
   pooled_s[i] + b[e]) so R_s is one tensor_scalar_mul of a [128, 16] mask by
   the per-partition scalar rwT.
 - conv1/conv3 are 1x1 convs = matmuls over the 784 pixels; conv2 (3x3,
   pad 1) = 9 accumulating matmuls with shifted access patterns over a
   zero-padded [128, 30, 30] tile.
 - Residual: identity matmul accumulates x into the conv3 PSUM; bias+relu
   applied in the drains.
 - Engine balance per sample: PE 7.8us (convs+combines+residual+router),
   DVE ~6.9us (2 reduces, combine drains, conv3 c=1 drains), ACT ~6.7us
   (2 reduces via accum_out, sigmoid, conv1/2 drains, conv3 c=0 drains),
   Pool (memset only; GPSIMD cannot touch PSUM and cannot reduce the free
   axis, so it cannot take real work).  Conv PSUM tiles stay small
   ([128, 14, 28], bufs=4) — deep buffering decouples the PE from drain
   latency; pairing banks into wider tiles was tried and made conv3
   drain-paced.
 - Emission is software-pipelined one sample ahead: slot s runs convs of s
   on the PE while emitting the combines of s+1 (drains overlap convs) and
   the router of s+2 at slot end (keeps reduces behind the drains in the
   in-order DVE/ACT queues).  DMA order: x0, cc, b1R, biases, b2 pieces,
   x1, b3R, then remaining x in sample order; outputs trail each sample.
"""

import sys

import ml_dtypes
import numpy as np

sys.path.insert(0, "/opt/trn_rl_repo")

import concourse.bacc as bacc
import concourse.bass as bass
import concourse.mybir as mybir
import concourse.tile as tile
from concourse import bass_utils

EPS = 1e-5

B = 64          # global batch
NCORES = 8
BS = B // NCORES  # samples per core
E = 8           # experts
CIN = 512
WID = 128       # bottleneck width
COUT = 512
H = 28
P = H * H       # 784 pixels
NCH = 392       # pixels per conv output chunk (14 rows)

NC1 = 32        # 16-col combine chunks for W1 (512 cols)
NC2 = 72        # for W2 (1152 cols)
NC3 = 32        # for W3 (512 cols)

F16 = mybir.dt.float16
F32 = mybir.dt.float32
F8 = mybir.dt.float8e4


def build_program():
    nc = bacc.Bacc("TRN2", target_bir_lowering=False, debug=False)

    # ---- DRAM I/O (per-core shapes) ----
    x_d = nc.dram_tensor("x", [BS, 4, 128, P], F16, kind="ExternalInput")
    b1_d = nc.dram_tensor("bank1", [128, NC1 // 2, 2, 128], F8, kind="ExternalInput")
    b2_d = nc.dram_tensor("bank2", [128, NC2 // 2, 2, 128], F8, kind="ExternalInput")
    b3_d = nc.dram_tensor("bank3", [128, NC3 // 2, 2, 128], F8, kind="ExternalInput")
    wbar_d = nc.dram_tensor("wbar", [128, 2176], F16, kind="ExternalInput")
    cc_d = nc.dram_tensor("cc", [128, 1408], F16, kind="ExternalInput")
    biases_d = nc.dram_tensor("biases", [128, 6], F32, kind="ExternalInput")
    out_d = nc.dram_tensor("out", [BS, 4, 128, P], F16, kind="ExternalOutput")

    Relu = mybir.ActivationFunctionType.Relu
    Sigmoid = mybir.ActivationFunctionType.Sigmoid
    Copy = mybir.ActivationFunctionType.Copy
    ADD = mybir.AluOpType.add
    MAX = mybir.AluOpType.max

    with tile.TileContext(nc) as tc:
        with (
            tc.tile_pool(name="const", bufs=1) as constp,
            tc.tile_pool(name="xin", bufs=6) as xp,
            tc.tile_pool(name="xsplit", bufs=1) as xsp,
            tc.tile_pool(name="comb", bufs=2) as combp,
            tc.tile_pool(name="act", bufs=4) as actp,
            tc.tile_pool(name="small", bufs=4) as smallp,
            tc.tile_pool(name="rmat", bufs=6) as rp,
            tc.tile_pool(name="resid", bufs=4) as residp,
            tc.tile_pool(name="pscomb", bufs=2, space=bass.MemorySpace.PSUM) as pscombp,
            tc.tile_pool(name="psconv", bufs=5, space=bass.MemorySpace.PSUM) as psconvp,
            tc.tile_pool(name="psr", bufs=1, space=bass.MemorySpace.PSUM) as psrp,
        ):
            # ---- persistent constants ----
            # banks: [(e,j16), tc32, jt, i], 8x scaled fp8 (DoubleRow k-tiles)
            bank1 = constp.tile([128, NC1 // 2, 2, 128], F8)
            bank2 = constp.tile([128, NC2 // 2, 2, 128], F8)
            bank3 = constp.tile([128, NC3 // 2, 2, 128], F8)
            wbar = constp.tile([128, 2176], F16)       # 0.5*sum_e bank_e
            cc = constp.tile([128, 1408], F16)
            rw4 = cc[:, 0:512]        # RW4[i, (it,(e,j))]
            mask32 = cc[:, 512:576]   # [(e,j16),(jt,j')] = 16*delta_{16jt+j16,j'}
            ident = cc[:, 576:704]    # I128 for the residual matmul
            rbT = cc[0:1, 704:832]    # row 0: router_b[(e,j)]
            maskneg = cc[:, 832:896]  # -8*delta pattern
            wbar1 = cc[:, 896:1408]   # wbar cols 0:512 (rides the cc DMA)
            ones1 = constp.tile([1, 1], F16)
            biases = constp.tile([128, 6], F32)
            bias1 = biases[:, 0:1]
            bias2 = biases[:, 1:2]
            bias3 = biases[:, 2:6]

            nc.gpsimd.memset(ones1[:], 1.0)

            # DMA issue order = device service order. Sample 0's tiles and
            # the banks go first (they gate sample 0's convs); later samples'
            # x tiles follow the banks in need-order.
            xs_l, r_l = [], []
            for s in (0, 1):
                xs_l.append([
                    xsp.tile([128, P], F16, tag=f"xs{s}_{t}", name=f"xs{s}_{t}")
                    for t in range(4)
                ])
            xs0 = xs_l[0]
            for s in range(2, BS):
                big = xp.tile([128, 4 * P], F16, tag="xs", name=f"xs{s}")
                xs_l.append([big[:, t * P : (t + 1) * P] for t in range(4)])
            nc.sync.dma_start(xs0[0][:], x_d[0, 0])
            nc.sync.dma_start(xs0[1][:], x_d[0, 1])
            nc.sync.dma_start(xs0[2][:], x_d[0, 2])
            nc.sync.dma_start(xs0[3][:], x_d[0, 3])
            nc.sync.dma_start(cc[:], cc_d[:])
            nc.sync.dma_start(xs_l[1][0][:], x_d[1, 0])
            nc.sync.dma_start(xs_l[1][1][:], x_d[1, 1])
            nc.sync.dma_start(xs_l[1][2][:], x_d[1, 2])
            nc.sync.dma_start(xs_l[1][3][:], x_d[1, 3])
            nc.sync.dma_start(bank1[:], b1_d[:])
            nc.sync.dma_start(biases[:], biases_d[:])
            nc.sync.dma_start(bank2[:, 0:16], b2_d[:, 0:16])
            nc.sync.dma_start(wbar[:, 512:2176], wbar_d[:, 512:2176])
            nc.sync.dma_start(bank2[:, 16:32], b2_d[:, 16:32])
            nc.sync.dma_start(bank2[:, 32:36], b2_d[:, 32:36])
            nc.sync.dma_start(
                xs_l[2][0].tensor[:, :], x_d[2].transpose([1, 0, 2])
            )
            nc.sync.dma_start(bank3[:], b3_d[:])
            for s in range(3, BS):
                nc.sync.dma_start(
                    xs_l[s][0].tensor[:, :], x_d[s].transpose([1, 0, 2])
                )

            # ================= routers =====================================
            pooled_l = {}

            def emit_router_reduce(s):
                xs = xs_l[s]

                # Pooling split across DVE and ACT (accum_out).  For the
                # front samples (tiles landing 558ns apart) 3 DVE / 1 ACT
                # minimizes the latency to R; in steady state 2/2 balances
                # engine load.
                dve_tiles = (0, 1, 3) if s < 2 else (0, 1)
                pooled = smallp.tile([128, 4], F32, tag="pooled")
                for t in range(4):
                    if t in dve_tiles:
                        nc.vector.tensor_reduce(
                            pooled[:, t : t + 1],
                            xs[t][:, :],
                            axis=mybir.AxisListType.X,
                            op=ADD,
                        )
                    else:
                        scratch = smallp.tile([128, P], F16, tag="scratch")
                        nc.scalar.activation(
                            scratch[:],
                            xs[t][:, :],
                            Copy,
                            accum_out=pooled[:, t : t + 1],
                        )
                pooled_l[s] = pooled

            def emit_router_finish(s):
                pooled = pooled_l.pop(s)
                pooled16 = smallp.tile([128, 4], F16, tag="pooled16")
                nc.vector.tensor_copy(pooled16[:], pooled[:])

                rpsum = psrp.tile([128, 1], F32, tag="rpsum")
                for t in range(4):
                    nc.tensor.matmul(
                        rpsum[:],
                        rw4[:, t * 128 : (t + 1) * 128],
                        pooled16[:, t : t + 1],
                        start=(t == 0),
                        stop=False,
                    )
                nc.tensor.matmul(
                    rpsum[:],
                    rbT,
                    ones1[:],
                    start=False,
                    stop=True,
                )
                rwT = smallp.tile([128, 1], F32, tag="rwT", name=f"rwT{s}")
                nc.scalar.activation(rwT[:], rpsum[:], Sigmoid)

                rs = rp.tile([128, 2, 32], F8, tag="rs", name=f"rs{s}")
                nc.vector.scalar_tensor_tensor(
                    rs[:].rearrange("p t j -> p (t j)"),
                    mask32, rwT[:, 0:1], maskneg,
                    op0=mybir.AluOpType.mult, op1=ADD,
                )
                r_l.append(rs)

            def emit_router(s):
                emit_router_reduce(s)
                emit_router_finish(s)

            # ============ combines (bank chunks stationary) =================
            w_l = {}

            def combine(s, bank, a0, ncols, dst, d0, woff):
                # ncols 32-col chunks; DoubleRow fp8 matmuls contract 256
                # rows ((e, j in 32)) at 0.5 cycles/row
                rs = r_l[s]
                psc = pscombp.tile([128, 512], F32, tag="psc", name="psc")
                for a in range(ncols):
                    nc.tensor.matmul(
                        psc[:, 32 * a : 32 * a + 32],
                        bank[:, a0 + a, :, :],
                        rs[:],
                        start=True,
                        stop=True,
                        perf_mode=mybir.MatmulPerfMode.DoubleRow,
                    )
                # dst = wbar + psc/128  (the drain applies the fp8 scale
                # normalization and adds the expert-mean weights)
                wsrc = wbar1 if woff is None else wbar[:, woff + d0 : woff + d0 + 32 * ncols]
                if woff is None:
                    wsrc = wbar1[:, d0 : d0 + 32 * ncols]
                nc.vector.scalar_tensor_tensor(
                    dst[:, d0 : d0 + 32 * ncols],
                    psc[:, : 32 * ncols],
                    1.0 / 128.0,
                    wsrc,
                    op0=mybir.AluOpType.mult,
                    op1=ADD,
                )

            def emit_cw1(s):
                w1c = combp.tile([128, 512], F16, tag="w1c", name=f"w1c{s}")
                combine(s, bank1, 0, 16, w1c, 0, None)
                w_l.setdefault(s, {})["w1"] = w1c

            def emit_cw2(s):
                w2c = combp.tile([128, 1152], F16, tag="w2c", name=f"w2c{s}")
                combine(s, bank2, 0, 16, w2c, 0, 512)
                combine(s, bank2, 16, 16, w2c, 512, 512)
                combine(s, bank2, 32, 4, w2c, 1024, 512)
                w_l.setdefault(s, {})["w2"] = w2c

            def emit_cw3(s):
                w3c = combp.tile([128, 512], F16, tag="w3c", name=f"w3c{s}")
                combine(s, bank3, 0, 16, w3c, 0, 1664)
                w_l.setdefault(s, {})["w3"] = w3c

            # ============ convs =============================================
            def emit_conv1(s):
                xs = xs_l[s]
                w1c = w_l[s]["w1"]
                mid1 = actp.tile([128, 30, 30], F16, tag="mid1", name=f"mid1_{s}")
                # only the border needs zeroing (interior is overwritten by
                # the conv1 drain)
                nc.gpsimd.memset(mid1[:, 0, :], 0.0)
                nc.gpsimd.memset(mid1[:, 29, :], 0.0)
                nc.gpsimd.memset(mid1[:, 1:29, 0:1], 0.0)
                nc.gpsimd.memset(mid1[:, 1:29, 29:30], 0.0)
                for c in range(2):
                    ps1 = psconvp.tile([128, 14, 28], F32, tag="convps")
                    for k in range(4):
                        nc.tensor.matmul(
                            ps1[:],
                            w1c[:, k * 128 : (k + 1) * 128],
                            xs[k][:, c * NCH : (c + 1) * NCH],
                            start=(k == 0),
                            stop=(k == 3),
                        )
                    nc.scalar.activation(
                        mid1[:, 14 * c + 1 : 14 * c + 15, 1:29],
                        ps1[:],
                        Relu,
                        bias=bias1,
                    )
                w_l[s]["mid1"] = mid1

            def emit_conv2(s):
                mid1 = w_l[s]["mid1"]
                w2c = w_l[s]["w2"]
                out2 = actp.tile([128, P], F16, tag="out2", name=f"out2_{s}")
                for c in range(2):
                    ps2 = psconvp.tile([128, 14, 28], F32, tag="convps")
                    idx = 0
                    for dy in range(3):
                        for dx in range(3):
                            nc.tensor.matmul(
                                ps2[:],
                                w2c[:, (dy * 3 + dx) * 128 : (dy * 3 + dx + 1) * 128],
                                mid1[:, 14 * c + dy : 14 * c + dy + 14, dx : dx + 28],
                                start=(idx == 0),
                                stop=(idx == 8),
                            )
                            idx += 1
                    nc.scalar.activation(
                        out2[:, c * NCH : (c + 1) * NCH], ps2[:], Relu, bias=bias2
                    )
                w_l[s]["out2"] = out2

            def emit_conv3(s):
                xs = xs_l[s]
                out2 = w_l[s]["out2"]
                w3c = w_l[s]["w3"]
                ofull = actp.tile([128, 4 * P], F16, tag="ofull", name=f"of{s}")
                for m in range(4):
                    for c in range(2):
                        ps3 = psconvp.tile([128, 14, 28], F32, tag="convps")
                        nc.tensor.matmul(
                            ps3[:],
                            w3c[:, m * 128 : (m + 1) * 128],
                            out2[:, c * NCH : (c + 1) * NCH],
                            start=True,
                            stop=(c == 1 and s < BS - 1),
                        )
                        dst = ofull[:, m * P + c * NCH : m * P + (c + 1) * NCH]
                        if c == 1 and s < BS - 1:
                            # residual via DVE STT + relu on the idle Pool
                            # engine (no PE identity matmul for this chunk)
                            u = residp.tile([128, NCH], F16, tag="u")
                            nc.vector.scalar_tensor_tensor(
                                u[:],
                                xs[m][:, c * NCH : (c + 1) * NCH],
                                bias3[:, m : m + 1],
                                ps3[:].rearrange("p a b -> p (a b)"),
                                op0=ADD,
                                op1=ADD,
                            )
                            nc.gpsimd.tensor_scalar_max(dst, u[:], 0.0)
                        else:
                            # residual via PE identity matmul; drain on ACT
                            # (DVE tensor_scalar for the last sample's c=1
                            # chunks, so the teardown drains run in parallel)
                            nc.tensor.matmul(
                                ps3[:],
                                ident,
                                xs[m][:, c * NCH : (c + 1) * NCH],
                                start=False,
                                stop=True,
                            )
                            if c == 1:
                                nc.vector.tensor_scalar(
                                    dst,
                                    ps3[:].rearrange("p a b -> p (a b)"),
                                    bias3[:, m : m + 1],
                                    0.0,
                                    op0=ADD,
                                    op1=MAX,
                                )
                            else:
                                nc.scalar.activation(
                                    dst,
                                    ps3[:].rearrange("p a b -> p (a b)"),
                                    Relu,
                                    bias=bias3[:, m : m + 1],
                                )
                    nc.sync.dma_start(
                        out_d[s, m], ofull[:, m * P : (m + 1) * P]
                    )
                del w_l[s]

            # ============ schedule ==========================================
            # Prelude: samples 0 AND 1 thread their conv1/conv2 between bank
            # arrivals, so the pipeline exits the DMA-bound front two samples
            # deep.  Router 2's reduces are issued as soon as x2 lands (it
            # arrives just before b3R) and finished after conv3(1), keeping
            # both the reduce latency and the in-order queues off the PE's
            # critical path.
            emit_router(0)
            emit_cw1(0)
            emit_conv1(0)
            emit_router(1)
            emit_cw1(1)
            emit_cw2(0)
            emit_conv1(1)
            emit_cw2(1)
            emit_conv2(0)
            emit_conv2(1)
            emit_cw3(0)
            emit_cw3(1)
            emit_router(2)
            emit_conv3(0)
            emit_router(3)
            emit_cw1(2)
            emit_cw2(2)
            emit_conv3(1)
            emit_cw3(2)
            emit_router(4)
            # Steady slots: convs of s interleaved with the combines of s+1
            # (their drains overlap the conv matmuls); router of s+3 last —
            # the ~2.3us reduce+sigmoid latency then has a full slot of
            # slack before cw1(s+3) consumes R, instead of stalling the PE
            # at each slot boundary.
            for s in range(2, BS):
                emit_conv1(s)
                if s + 1 < BS:
                    emit_cw1(s + 1)
                    emit_cw2(s + 1)
                emit_conv2(s)
                if s + 1 < BS:
                    emit_cw3(s + 1)
                emit_conv3(s)
                if s + 3 < BS:
                    emit_router(s + 3)

    nc.compile()
    return nc


_NC_CACHE = None


def _get_program():
    global _NC_CACHE
    if _NC_CACHE is None:
        _NC_CACHE = build_program()
    return _NC_CACHE


def prepare_inputs(
    x, router_w, router_b, w1, w2, w3,
    g1, b1, m1, v1, g2, b2, m2, v2, g3, b3, m3, v3,
):
    """Host-side preprocessing -> per-core in_maps."""
    f = np.float32
    x = np.asarray(x, f)
    router_w = np.asarray(router_w, f)
    router_b = np.asarray(router_b, f)
    w1 = np.asarray(w1, f)
    w2 = np.asarray(w2, f)
    w3 = np.asarray(w3, f)

    s1 = np.asarray(g1, f) / np.sqrt(np.asarray(v1, f) + EPS)
    s2 = np.asarray(g2, f) / np.sqrt(np.asarray(v2, f) + EPS)
    s3 = np.asarray(g3, f) / np.sqrt(np.asarray(v3, f) + EPS)
    bb1 = np.asarray(b1, f) - np.asarray(m1, f) * s1
    bb2 = np.asarray(b2, f) - np.asarray(m2, f) * s2
    bb3 = np.asarray(b3, f) - np.asarray(m3, f) * s3

    # Combined-weight layouts (per expert), matching the device tiles:
    #  Wb1[e, p, it*128+o] = w1s[e, o, it*128+p]
    w1s = w1[:, :, :, 0, 0] * s1[None, :, None]            # [E, o=128, i=512]
    Wb1 = (
        w1s.reshape(E, 128, 4, 128).transpose(0, 3, 2, 1).reshape(E, 128, 512)
    )
    #  Wb2[e, ci, tap*128+o]
    w2s = w2 * s2[None, :, None, None, None]               # [E, o, ci, dy, dx]
    Wb2 = (
        w2s.transpose(0, 3, 4, 2, 1).reshape(E, 9, 128, 128)
        .transpose(0, 2, 1, 3).reshape(E, 128, 1152)
    )
    #  Wb3[e, ci, o]
    w3s = w3[:, :, :, 0, 0] * s3[None, :, None]            # [E, o=512, ci=128]
    Wb3 = w3s.transpose(0, 2, 1)                           # [E, 128, 512]

    def to_bankR(Wb, ncols):
        # [E, 128(i), C] -> [(e,j16), (tc32, jt, i)] with C = 16*ncols.
        # Banks are 8x-scaled and stored fp8e4m3: they only carry the
        # per-sample DELTA sum_e 16*(rw_e-0.5) * bank_e (the expert-mean
        # wbar is added in fp16 at drain time), so fp8 quantization error
        # lands on a ~2% correction term.  jt is the DoubleRow k-tile.
        return np.ascontiguousarray(
            (Wb * 8.0).reshape(E, 128, ncols // 2, 2, 16)
            .transpose(0, 4, 2, 3, 1)          # [e, j16, tc32, jt, i]
            .reshape(128, -1)
        ).astype(ml_dtypes.float8_e4m3fn)

    bank1 = to_bankR(Wb1, NC1)
    bank2 = to_bankR(Wb2, NC2)
    bank3 = to_bankR(Wb3, NC3)
    wbar = np.concatenate(
        [0.5 * Wb1.sum(axis=0), 0.5 * Wb2.sum(axis=0), 0.5 * Wb3.sum(axis=0)],
        axis=1,
    ).astype(np.float16)

    cc = np.zeros((128, 1408), np.float16)
    # RW4[i, it*128 + (e*16+j)] = router_w[e, it*128+i] / P
    rwt4 = (router_w / float(P)).reshape(E, 4, 128)        # [e, it, i]
    arr = np.repeat(rwt4.transpose(1, 2, 0)[:, :, :, None], 16, axis=3)
    cc[:, 0:512] = arr.reshape(4, 128, 128).transpose(1, 0, 2).reshape(128, 512)
    # pat[(e,j16), (jt,j')] = delta_{j' == 16*jt + j16}
    pat = np.tile(
        np.eye(32, dtype=np.float16).reshape(2, 16, 32).transpose(1, 0, 2)
        .reshape(16, 64),
        (8, 1),
    )
    cc[:, 512:576] = 16.0 * pat
    cc[:, 576:704] = np.eye(128, dtype=np.float16)
    cc[0, 704:832] = np.repeat(router_b.astype(np.float16), 16)
    cc[:, 832:896] = -8.0 * pat
    cc[:, 896:1408] = wbar[:, 0:512]

    biases = np.concatenate(
        [bb1.reshape(128, 1), bb2.reshape(128, 1), bb3.reshape(4, 128).T],
        axis=1,
    ).astype(np.float32)

    x16 = x.reshape(B, 4, 128, P).astype(np.float16)

    shared = {
        "bank1": bank1,
        "bank2": bank2,
        "bank3": bank3,
        "wbar": wbar,
        "cc": cc,
        "biases": biases,
    }
    in_maps = []
    for c in range(NCORES):
        m = dict(shared)
        m["x"] = np.ascontiguousarray(x16[c * BS : (c + 1) * BS])
        in_maps.append(m)
    return in_maps


def run(in_maps, trace=False, tmpdir=None):
    nc = _get_program()
    res = bass_utils.run_bass_kernel_spmd(
        nc, in_maps, core_ids=list(range(NCORES)), trace=trace, tmpdir=tmpdir
    )
    outs = [np.asarray(r["out"], np.float32) for r in res.results]
    full = np.concatenate(outs, axis=0).reshape(B, CIN, H, H)
    return full, res


def kernel(**inputs):
    in_maps = prepare_inputs(**inputs)
    full, _ = run(in_maps, trace=False)
    return full


# revision 109
# speedup vs baseline: 1.0211x; 1.0014x over previous
"""
Bass/Trainium2 kernel for nn_BottleneckShared (moe_routing).

Computation (per sample b):
    rw   = sigmoid(mean_hw(x) @ router_w.T + router_b)          # [E]
    Wk_b = sum_e rw[e] * wk[e]            (k = 1,2,3)           # per-sample conv kernels
    out  = relu(bn3(conv3(relu(bn2(conv2(relu(bn1(conv1(x)))))))) + x)

Sharding: data-parallel over batch. 64 samples -> 8 NeuronCores x 8 samples.
Expert banks / router / BN params are replicated to every core.

Device-side design notes (v3):
 - BN scales folded into the expert banks on the host; BN biases applied in
   the PSUM-drain of each conv.
 - Weight combine via a block-diagonal router matrix streamed against
   STATIONARY bank chunks: bank chunk tc is a [128, 128] tile whose partition
   (e, j in 16) holds bank_e[:, 16*tc + j]; the moving operand is
   R_s[(e,j), j'] = rw_se * delta_jj' ([128, 16]).  Each matmul produces 16
   combined-weight columns directly:
     out[i, j'] = sum_{e,j} bank_e[i, 16tc+j] * rw_se delta_jj'
                = sum_e rw_se bank_e[i, 16tc+j'].
   PE cost is 16 columns per chunk (2176 columns/sample) vs 512 per expert
   (17408/sample) for the scaled-identity formulation.
 - Router computed transposed: rwT[(e,j), s] = sigmoid(sum_i RW4[i,(e,j)] *
   pooled_s[i] + b[e]) so R_s is one tensor_scalar_mul of a [128, 16] mask by
   the per-partition scalar rwT.
 - conv1/conv3 are 1x1 convs = matmuls over the 784 pixels; conv2 (3x3,
   pad 1) = 9 accumulating matmuls with shifted access patterns over a
   zero-padded [128, 30, 30] tile.
 - Residual: identity matmul accumulates x into the conv3 PSUM; bias+relu
   applied in the drains.
 - Engine balance per sample: PE 7.8us (convs+combines+residual+router),
   DVE ~6.9us (2 reduces, combine drains, conv3 c=1 drains), ACT ~6.7us
   (2 reduces via accum_out, sigmoid, conv1/2 drains, conv3 c=0 drains),
   Pool (memset only; GPSIMD cannot touch PSUM and cannot reduce the free
   axis, so it cannot take real work).  Conv PSUM tiles stay small
   ([128, 14, 28], bufs=4) — deep buffering decouples the PE from drain
   latency; pairing banks into wider tiles was tried and made conv3
   drain-paced.
 - Emission is software-pipelined one sample ahead: slot s runs convs of s
   on the PE while emitting the combines of s+1 (drains overlap convs) and
   the router of s+2 at slot end (keeps reduces behind the drains in the
   in-order DVE/ACT queues).  DMA order: x0, cc, b1R, biases, b2 pieces,
   x1, b3R, then remaining x in sample order; outputs trail each sample.
"""

import sys

import ml_dtypes
import numpy as np

sys.path.insert(0, "/opt/trn_rl_repo")

import concourse.bacc as bacc
import concourse.bass as bass
import concourse.mybir as mybir
import concourse.tile as tile
from concourse import bass_utils

EPS = 1e-5

B = 64          # global batch
NCORES = 8
BS = B // NCORES  # samples per core
E = 8           # experts
CIN = 512
WID = 128       # bottleneck width
COUT = 512
H = 28
P = H * H       # 784 pixels
NCH = 392       # pixels per conv output chunk (14 rows)

NC1 = 32        # 16-col combine chunks for W1 (512 cols)
NC2 = 72        # for W2 (1152 cols)
NC3 = 32        # for W3 (512 cols)

F16 = mybir.dt.float16
F32 = mybir.dt.float32
F8 = mybir.dt.float8e4


def build_program():
    nc = bacc.Bacc("TRN2", target_bir_lowering=False, debug=False)

    # ---- DRAM I/O (per-core shapes) ----
    x_d = nc.dram_tensor("x", [BS, 4, 128, P], F16, kind="ExternalInput")
    b1_d = nc.dram_tensor("bank1", [128, NC1 // 2, 2, 128], F8, kind="ExternalInput")
    b2_d = nc.dram_tensor("bank2", [128, NC2 // 2, 2, 128], F8, kind="ExternalInput")
    b3_d = nc.dram_tensor("bank3", [128, NC3 // 2, 2, 128], F8, kind="ExternalInput")
    wbar_d = nc.dram_tensor("wbar", [128, 2176], F16, kind="ExternalInput")
    cc_d = nc.dram_tensor("cc", [128, 1416], F16, kind="ExternalInput")
    out_d = nc.dram_tensor("out", [BS, 4, 128, P], F16, kind="ExternalOutput")

    Relu = mybir.ActivationFunctionType.Relu
    Sigmoid = mybir.ActivationFunctionType.Sigmoid
    Copy = mybir.ActivationFunctionType.Copy
    ADD = mybir.AluOpType.add
    MAX = mybir.AluOpType.max

    with tile.TileContext(nc) as tc:
        with (
            tc.tile_pool(name="const", bufs=1) as constp,
            tc.tile_pool(name="xin", bufs=6) as xp,
            tc.tile_pool(name="xsplit", bufs=1) as xsp,
            tc.tile_pool(name="comb", bufs=2) as combp,
            tc.tile_pool(name="act", bufs=4) as actp,
            tc.tile_pool(name="small", bufs=4) as smallp,
            tc.tile_pool(name="rmat", bufs=6) as rp,
            tc.tile_pool(name="resid", bufs=4) as residp,
            tc.tile_pool(name="pscomb", bufs=2, space=bass.MemorySpace.PSUM) as pscombp,
            tc.tile_pool(name="psconv", bufs=5, space=bass.MemorySpace.PSUM) as psconvp,
            tc.tile_pool(name="psr", bufs=1, space=bass.MemorySpace.PSUM) as psrp,
        ):
            # ---- persistent constants ----
            # banks: [(e,j16), tc32, jt, i], 8x scaled fp8 (DoubleRow k-tiles)
            bank1 = constp.tile([128, NC1 // 2, 2, 128], F8)
            bank2 = constp.tile([128, NC2 // 2, 2, 128], F8)
            bank3 = constp.tile([128, NC3 // 2, 2, 128], F8)
            wbar = constp.tile([128, 2176], F16)       # 0.5*sum_e bank_e
            cc = constp.tile([128, 1416], F16)
            rw4 = cc[:, 0:512]        # RW4[i, (it,(e,j))]
            mask32 = cc[:, 512:576]   # [(e,j16),(jt,j')] = 16*delta_{16jt+j16,j'}
            ident = cc[:, 576:704]    # I128 for the residual matmul
            rbT = cc[0:1, 704:832]    # row 0: router_b[(e,j)]
            maskneg = cc[:, 832:896]  # -8*delta pattern
            wbar1 = cc[:, 896:1408]   # wbar cols 0:512 (rides the cc DMA)
            # BN biases ride the cc transfer in fp16 (error ~1e-4 relative)
            # and are widened to fp32 on-device with one tiny DVE copy (DVE
            # scalar operands must be fp32): zero extra front DMA slots
            biases = constp.tile([128, 6], F32)
            bias1 = biases[:, 0:1]
            bias2 = biases[:, 1:2]
            bias3 = biases[:, 2:6]
            ones1 = cc[0:1, 1414:1415]

            # DMA issue order = device service order. Sample 0's tiles and
            # the banks go first (they gate sample 0's convs); later samples'
            # x tiles follow the banks in need-order.
            xs_l, r_l = [], []
            for s in (0, 1):
                xs_l.append([
                    xsp.tile([128, P], F16, tag=f"xs{s}_{t}", name=f"xs{s}_{t}")
                    for t in range(4)
                ])
            xs0 = xs_l[0]
            for s in range(2, BS):
                big = xp.tile([128, 4 * P], F16, tag="xs", name=f"xs{s}")
                xs_l.append([big[:, t * P : (t + 1) * P] for t in range(4)])
            nc.sync.dma_start(xs0[0][:], x_d[0, 0])
            nc.sync.dma_start(xs0[1][:], x_d[0, 1])
            nc.sync.dma_start(xs0[2][:], x_d[0, 2])
            nc.sync.dma_start(xs0[3][:], x_d[0, 3])
            nc.sync.dma_start(cc[:], cc_d[:])
            nc.sync.dma_start(xs_l[1][0][:], x_d[1, 0])
            nc.sync.dma_start(xs_l[1][1][:], x_d[1, 1])
            nc.sync.dma_start(xs_l[1][2][:], x_d[1, 2])
            nc.sync.dma_start(xs_l[1][3][:], x_d[1, 3])
            nc.vector.tensor_copy(biases[:], cc[:, 1408:1414])
            nc.sync.dma_start(bank1[:], b1_d[:])
            nc.sync.dma_start(bank2[:, 0:16], b2_d[:, 0:16])
            nc.sync.dma_start(wbar[:, 512:2176], wbar_d[:, 512:2176])
            nc.sync.dma_start(bank2[:, 16:32], b2_d[:, 16:32])
            nc.sync.dma_start(bank2[:, 32:36], b2_d[:, 32:36])
            nc.sync.dma_start(
                xs_l[2][0].tensor[:, :], x_d[2].transpose([1, 0, 2])
            )
            nc.sync.dma_start(bank3[:], b3_d[:])
            for s in range(3, BS):
                nc.sync.dma_start(
                    xs_l[s][0].tensor[:, :], x_d[s].transpose([1, 0, 2])
                )

            # ================= routers =====================================
            pooled_l = {}

            def emit_router_reduce(s):
                xs = xs_l[s]

                # Pooling split across DVE and ACT (accum_out).  For the
                # front samples (tiles landing 558ns apart) 3 DVE / 1 ACT
                # minimizes the latency to R; in steady state 2/2 balances
                # engine load.
                dve_tiles = (0, 1, 3) if s < 2 else (0, 1)
                pooled = smallp.tile([128, 4], F32, tag="pooled")
                for t in range(4):
                    if t in dve_tiles:
                        nc.vector.tensor_reduce(
                            pooled[:, t : t + 1],
                            xs[t][:, :],
                            axis=mybir.AxisListType.X,
                            op=ADD,
                        )
                    else:
                        scratch = smallp.tile([128, P], F16, tag="scratch")
                        nc.scalar.activation(
                            scratch[:],
                            xs[t][:, :],
                            Copy,
                            accum_out=pooled[:, t : t + 1],
                        )
                pooled_l[s] = pooled

            def emit_router_finish(s):
                pooled = pooled_l.pop(s)
                pooled16 = smallp.tile([128, 4], F16, tag="pooled16")
                nc.vector.tensor_copy(pooled16[:], pooled[:])

                rpsum = psrp.tile([128, 1], F32, tag="rpsum")
                for t in range(4):
                    nc.tensor.matmul(
                        rpsum[:],
                        rw4[:, t * 128 : (t + 1) * 128],
                        pooled16[:, t : t + 1],
                        start=(t == 0),
                        stop=False,
                    )
                nc.tensor.matmul(
                    rpsum[:],
                    rbT,
                    ones1,
                    start=False,
                    stop=True,
                )
                rwT = smallp.tile([128, 1], F32, tag="rwT", name=f"rwT{s}")
                nc.scalar.activation(rwT[:], rpsum[:], Sigmoid)

                rs = rp.tile([128, 2, 32], F8, tag="rs", name=f"rs{s}")
                nc.vector.scalar_tensor_tensor(
                    rs[:].rearrange("p t j -> p (t j)"),
                    mask32, rwT[:, 0:1], maskneg,
                    op0=mybir.AluOpType.mult, op1=ADD,
                )
                r_l.append(rs)

            def emit_router(s):
                emit_router_reduce(s)
                emit_router_finish(s)

            # ============ combines (bank chunks stationary) =================
            w_l = {}

            def combine(s, bank, a0, ncols, dst, d0, woff):
                # ncols 32-col chunks; DoubleRow fp8 matmuls contract 256
                # rows ((e, j in 32)) at 0.5 cycles/row
                rs = r_l[s]
                psc = pscombp.tile([128, 512], F32, tag="psc", name="psc")
                for a in range(ncols):
                    nc.tensor.matmul(
                        psc[:, 32 * a : 32 * a + 32],
                        bank[:, a0 + a, :, :],
                        rs[:],
                        start=True,
                        stop=True,
                        perf_mode=mybir.MatmulPerfMode.DoubleRow,
                    )
                # dst = wbar + psc/128  (the drain applies the fp8 scale
                # normalization and adds the expert-mean weights)
                wsrc = wbar1 if woff is None else wbar[:, woff + d0 : woff + d0 + 32 * ncols]
                if woff is None:
                    wsrc = wbar1[:, d0 : d0 + 32 * ncols]
                nc.vector.scalar_tensor_tensor(
                    dst[:, d0 : d0 + 32 * ncols],
                    psc[:, : 32 * ncols],
                    1.0 / 128.0,
                    wsrc,
                    op0=mybir.AluOpType.mult,
                    op1=ADD,
                )

            def emit_cw1(s):
                w1c = combp.tile([128, 512], F16, tag="w1c", name=f"w1c{s}")
                combine(s, bank1, 0, 16, w1c, 0, None)
                w_l.setdefault(s, {})["w1"] = w1c

            def emit_cw2(s):
                w2c = combp.tile([128, 1152], F16, tag="w2c", name=f"w2c{s}")
                combine(s, bank2, 0, 16, w2c, 0, 512)
                combine(s, bank2, 16, 16, w2c, 512, 512)
                combine(s, bank2, 32, 4, w2c, 1024, 512)
                w_l.setdefault(s, {})["w2"] = w2c

            def emit_cw3(s):
                w3c = combp.tile([128, 512], F16, tag="w3c", name=f"w3c{s}")
                combine(s, bank3, 0, 16, w3c, 0, 1664)
                w_l.setdefault(s, {})["w3"] = w3c

            # ============ convs =============================================
            def emit_conv1(s):
                xs = xs_l[s]
                w1c = w_l[s]["w1"]
                mid1 = actp.tile([128, 30, 30], F16, tag="mid1", name=f"mid1_{s}")
                # only the border needs zeroing (interior is overwritten by
                # the conv1 drain)
                nc.gpsimd.memset(mid1[:, 0, :], 0.0)
                nc.gpsimd.memset(mid1[:, 29, :], 0.0)
                nc.gpsimd.memset(mid1[:, 1:29, 0:1], 0.0)
                nc.gpsimd.memset(mid1[:, 1:29, 29:30], 0.0)
                for c in range(2):
                    ps1 = psconvp.tile([128, 14, 28], F32, tag="convps")
                    for k in range(4):
                        nc.tensor.matmul(
                            ps1[:],
                            w1c[:, k * 128 : (k + 1) * 128],
                            xs[k][:, c * NCH : (c + 1) * NCH],
                            start=(k == 0),
                            stop=(k == 3),
                        )
                    nc.scalar.activation(
                        mid1[:, 14 * c + 1 : 14 * c + 15, 1:29],
                        ps1[:],
                        Relu,
                        bias=bias1,
                    )
                w_l[s]["mid1"] = mid1

            def emit_conv2(s):
                mid1 = w_l[s]["mid1"]
                w2c = w_l[s]["w2"]
                out2 = actp.tile([128, P], F16, tag="out2", name=f"out2_{s}")
                for c in range(2):
                    ps2 = psconvp.tile([128, 14, 28], F32, tag="convps")
                    idx = 0
                    for dy in range(3):
                        for dx in range(3):
                            nc.tensor.matmul(
                                ps2[:],
                                w2c[:, (dy * 3 + dx) * 128 : (dy * 3 + dx + 1) * 128],
                                mid1[:, 14 * c + dy : 14 * c + dy + 14, dx : dx + 28],
                                start=(idx == 0),
                                stop=(idx == 8),
                            )
                            idx += 1
                    nc.scalar.activation(
                        out2[:, c * NCH : (c + 1) * NCH], ps2[:], Relu, bias=bias2
                    )
                w_l[s]["out2"] = out2

            def emit_conv3(s):
                xs = xs_l[s]
                out2 = w_l[s]["out2"]
                w3c = w_l[s]["w3"]
                ofull = actp.tile([128, 4 * P], F16, tag="ofull", name=f"of{s}")
                for m in range(4):
                    for c in range(2):
                        ps3 = psconvp.tile([128, 14, 28], F32, tag="convps")
                        nc.tensor.matmul(
                            ps3[:],
                            w3c[:, m * 128 : (m + 1) * 128],
                            out2[:, c * NCH : (c + 1) * NCH],
                            start=True,
                            stop=(c == 1 and s < BS - 1),
                        )
                        dst = ofull[:, m * P + c * NCH : m * P + (c + 1) * NCH]
                        if c == 1 and s < BS - 1:
                            # residual via DVE STT + relu on the idle Pool
                            # engine (no PE identity matmul for this chunk)
                            u = residp.tile([128, NCH], F16, tag="u")
                            nc.vector.scalar_tensor_tensor(
                                u[:],
                                xs[m][:, c * NCH : (c + 1) * NCH],
                                bias3[:, m : m + 1],
                                ps3[:].rearrange("p a b -> p (a b)"),
                                op0=ADD,
                                op1=ADD,
                            )
                            nc.gpsimd.tensor_scalar_max(dst, u[:], 0.0)
                        else:
                            # residual via PE identity matmul; drain on ACT
                            # (DVE tensor_scalar for the last sample's c=1
                            # chunks, so the teardown drains run in parallel)
                            nc.tensor.matmul(
                                ps3[:],
                                ident,
                                xs[m][:, c * NCH : (c + 1) * NCH],
                                start=False,
                                stop=True,
                            )
                            if c == 1:
                                nc.vector.tensor_scalar(
                                    dst,
                                    ps3[:].rearrange("p a b -> p (a b)"),
                                    bias3[:, m : m + 1],
                                    0.0,
                                    op0=ADD,
                                    op1=MAX,
                                )
                            else:
                                nc.scalar.activation(
                                    dst,
                                    ps3[:].rearrange("p a b -> p (a b)"),
                                    Relu,
                                    bias=bias3[:, m : m + 1],
                                )
                    nc.sync.dma_start(
                        out_d[s, m], ofull[:, m * P : (m + 1) * P]
                    )
                del w_l[s]

            # ============ schedule ==========================================
            # Prelude: samples 0 AND 1 thread their conv1/conv2 between bank
            # arrivals, so the pipeline exits the DMA-bound front two samples
            # deep.  Router 2's reduces are issued as soon as x2 lands (it
            # arrives just before b3R) and finished after conv3(1), keeping
            # both the reduce latency and the in-order queues off the PE's
            # critical path.
            emit_router(0)
            emit_cw1(0)
            emit_conv1(0)
            emit_router(1)
            emit_cw1(1)
            emit_cw2(0)
            emit_conv1(1)
            emit_cw2(1)
            emit_conv2(0)
            emit_conv2(1)
            emit_cw3(0)
            emit_cw3(1)
            emit_router(2)
            emit_conv3(0)
            emit_router(3)
            emit_cw1(2)
            emit_cw2(2)
            emit_conv3(1)
            emit_cw3(2)
            emit_router(4)
            # Steady slots: convs of s interleaved with the combines of s+1
            # (their drains overlap the conv matmuls); router of s+3 last —
            # the ~2.3us reduce+sigmoid latency then has a full slot of
            # slack before cw1(s+3) consumes R, instead of stalling the PE
            # at each slot boundary.
            for s in range(2, BS):
                emit_conv1(s)
                if s + 1 < BS:
                    emit_cw1(s + 1)
                    emit_cw2(s + 1)
                emit_conv2(s)
                if s + 1 < BS:
                    emit_cw3(s + 1)
                emit_conv3(s)
                if s + 3 < BS:
                    emit_router(s + 3)

    nc.compile()
    return nc


_NC_CACHE = None


def _get_program():
    global _NC_CACHE
    if _NC_CACHE is None:
        _NC_CACHE = build_program()
    return _NC_CACHE


def prepare_inputs(
    x, router_w, router_b, w1, w2, w3,
    g1, b1, m1, v1, g2, b2, m2, v2, g3, b3, m3, v3,
):
    """Host-side preprocessing -> per-core in_maps."""
    f = np.float32
    x = np.asarray(x, f)
    router_w = np.asarray(router_w, f)
    router_b = np.asarray(router_b, f)
    w1 = np.asarray(w1, f)
    w2 = np.asarray(w2, f)
    w3 = np.asarray(w3, f)

    s1 = np.asarray(g1, f) / np.sqrt(np.asarray(v1, f) + EPS)
    s2 = np.asarray(g2, f) / np.sqrt(np.asarray(v2, f) + EPS)
    s3 = np.asarray(g3, f) / np.sqrt(np.asarray(v3, f) + EPS)
    bb1 = np.asarray(b1, f) - np.asarray(m1, f) * s1
    bb2 = np.asarray(b2, f) - np.asarray(m2, f) * s2
    bb3 = np.asarray(b3, f) - np.asarray(m3, f) * s3

    # Combined-weight layouts (per expert), matching the device tiles:
    #  Wb1[e, p, it*128+o] = w1s[e, o, it*128+p]
    w1s = w1[:, :, :, 0, 0] * s1[None, :, None]            # [E, o=128, i=512]
    Wb1 = (
        w1s.reshape(E, 128, 4, 128).transpose(0, 3, 2, 1).reshape(E, 128, 512)
    )
    #  Wb2[e, ci, tap*128+o]
    w2s = w2 * s2[None, :, None, None, None]               # [E, o, ci, dy, dx]
    Wb2 = (
        w2s.transpose(0, 3, 4, 2, 1).reshape(E, 9, 128, 128)
        .transpose(0, 2, 1, 3).reshape(E, 128, 1152)
    )
    #  Wb3[e, ci, o]
    w3s = w3[:, :, :, 0, 0] * s3[None, :, None]            # [E, o=512, ci=128]
    Wb3 = w3s.transpose(0, 2, 1)                           # [E, 128, 512]

    def to_bankR(Wb, ncols):
        # [E, 128(i), C] -> [(e,j16), (tc32, jt, i)] with C = 16*ncols.
        # Banks are 8x-scaled and stored fp8e4m3: they only carry the
        # per-sample DELTA sum_e 16*(rw_e-0.5) * bank_e (the expert-mean
        # wbar is added in fp16 at drain time), so fp8 quantization error
        # lands on a ~2% correction term.  jt is the DoubleRow k-tile.
        return np.ascontiguousarray(
            (Wb * 8.0).reshape(E, 128, ncols // 2, 2, 16)
            .transpose(0, 4, 2, 3, 1)          # [e, j16, tc32, jt, i]
            .reshape(128, -1)
        ).astype(ml_dtypes.float8_e4m3fn)

    bank1 = to_bankR(Wb1, NC1)
    bank2 = to_bankR(Wb2, NC2)
    bank3 = to_bankR(Wb3, NC3)
    wbar = np.concatenate(
        [0.5 * Wb1.sum(axis=0), 0.5 * Wb2.sum(axis=0), 0.5 * Wb3.sum(axis=0)],
        axis=1,
    ).astype(np.float16)

    cc = np.zeros((128, 1416), np.float16)
    # RW4[i, it*128 + (e*16+j)] = router_w[e, it*128+i] / P
    rwt4 = (router_w / float(P)).reshape(E, 4, 128)        # [e, it, i]
    arr = np.repeat(rwt4.transpose(1, 2, 0)[:, :, :, None], 16, axis=3)
    cc[:, 0:512] = arr.reshape(4, 128, 128).transpose(1, 0, 2).reshape(128, 512)
    # pat[(e,j16), (jt,j')] = delta_{j' == 16*jt + j16}
    pat = np.tile(
        np.eye(32, dtype=np.float16).reshape(2, 16, 32).transpose(1, 0, 2)
        .reshape(16, 64),
        (8, 1),
    )
    cc[:, 512:576] = 16.0 * pat
    cc[:, 576:704] = np.eye(128, dtype=np.float16)
    cc[0, 704:832] = np.repeat(router_b.astype(np.float16), 16)
    cc[:, 832:896] = -8.0 * pat
    cc[:, 896:1408] = wbar[:, 0:512]
    cc[:, 1408] = bb1
    cc[:, 1409] = bb2
    cc[:, 1410:1414] = bb3.reshape(4, 128).T
    cc[:, 1414] = 1.0



    x16 = x.reshape(B, 4, 128, P).astype(np.float16)

    shared = {
        "bank1": bank1,
        "bank2": bank2,
        "bank3": bank3,
        "wbar": wbar,
        "cc": cc,
    }
    in_maps = []
    for c in range(NCORES):
        m = dict(shared)
        m["x"] = np.ascontiguousarray(x16[c * BS : (c + 1) * BS])
        in_maps.append(m)
    return in_maps


def run(in_maps, trace=False, tmpdir=None):
    nc = _get_program()
    res = bass_utils.run_bass_kernel_spmd(
        nc, in_maps, core_ids=list(range(NCORES)), trace=trace, tmpdir=tmpdir
    )
    outs = [np.asarray(r["out"], np.float32) for r in res.results]
    full = np.concatenate(outs, axis=0).reshape(B, CIN, H, H)
    return full, res


def kernel(**inputs):
    in_maps = prepare_inputs(**inputs)
    full, _ = run(in_maps, trace=False)
    return full
